# revision 24
# baseline (speedup 1.0000x reference)
"""Bass/Trainium2 kernel for the 2-layer GAT (nn_GAT_11106785427688).

Strategy (8 NeuronCores, SPMD single NEFF):
- dst-ownership sharding: core c owns nodes [c*OWN, (c+1)*OWN); it receives
  every edge whose dst it owns (~137K edges), so segment-softmax denominators
  and message sums complete locally -- no all-reduce. One AllGather of the
  layer-1 activations between layers; host assembles the final output from
  per-core slices.
- Per-edge gather of packed [h | a_src.h] rows (fp16, 256B) from an HBM table
  via the SWDGE dma_gather custom op (int16 indices -> src buckets of 32768
  rows; table rows permuted so the dense phase writes 2KB-contiguous runs).
- No indexed scatter (HW dma_scatter_add loses duplicate updates): edges are
  grouped by 128-node dst window; the one-hot R [edges x nodes] and its
  transpose RT [nodes x edges] are PRECOMPUTED ON HOST (pure edge-index
  preprocessing) and streamed from HBM, so the DVE never builds one-hots.
  R turns segment-sum into PE matmul accumulated in PSUM; RT gathers the
  per-window a_dst values to edges via PE. Softmax division is deferred:
  out = (sum_e w*h[src]) / (sum_e w).
- exp(leakyrelu(e)) computed without max-subtraction (shift-invariant).
- adw_fill (self-loops + per-window a_dst) is interleaved with the dense
  table build so PE/Act/DVE/DMA overlap instead of running serial phases.
"""
import numpy as np
import ml_dtypes

from concourse import bacc, mybir
import concourse.tile as tile
from concourse.bass_utils import run_bass_kernel_spmd

# ---- problem constants ----
N = 100000
D = 64
H1, C1 = 4, 16
NEG = 0.2
NCORES = 8
OWN = 12544                 # 98 windows * 128 per core
BUCK = 32768
CHUNK = 1024                # gather idxs per dma_gather call (ring limit)
TPC = CHUNK // 128          # tiles per chunk = 8

F16 = mybir.dt.float16
F32 = mybir.dt.float32
BF16 = mybir.dt.bfloat16
I16 = mybir.dt.int16
NPF16 = np.float16
NPBF16 = ml_dtypes.bfloat16

ACT = mybir.ActivationFunctionType


def _derived():
    NW = OWN // 128
    NPAD = NCORES * OWN
    NBUCK = (NPAD + BUCK - 1) // BUCK
    TBL_ROWS = NBUCK * BUCK
    return NW, NPAD, NBUCK, TBL_ROWS


def _perm_row(src):
    """Permuted table row for node src: tb*1024 + p*8 + j (write-friendly)."""
    tb, r = np.divmod(src, 1024)
    j, p = np.divmod(r, 128)
    return tb * 1024 + p * 8 + j


def prep(edge_index):
    """Vectorized host prep: quantile-banded schedule.

    Per-(core,window) edges sorted by src, quantile-spread into the padded
    window group (G_w = roundup128(max-over-cores)). Window tiles are split
    into bands of <=3 tiles; the schedule is band-major so consecutive tiles
    cover the same src-quantile region. Each 1024-slot chunk then spans <=~31
    perm-blocks and gets ONE dma_gather call with a dynamic host-computed
    base (int16 idx). Bands are chunk-aligned (pad tiles trail per band).

    Also builds, per core, the fp16 one-hot streams R (edge-major: used as
    matmul lhsT for the per-window segment sums) and RT (node-major: used as
    lhsT to gather per-window a_dst values to edge positions).
    """
    NW, NPAD, NBUCK, TBL_ROWS = _derived()
    # self-loops are handled densely in adw_fill, not in the gather sweep
    src = np.asarray(edge_index[0])
    dst = np.asarray(edge_index[1])
    owner = dst // OWN

    per_core = []
    counts = np.zeros((NCORES, NW), np.int64)
    for c in range(NCORES):
        m = owner == c
        s = src[m]
        d = dst[m] - c * OWN
        w = d >> 7
        order = np.lexsort((s, w))
        s, d, w = s[order], d[order], w[order]
        per_core.append((s, d, w))
        counts[c] = np.bincount(w, minlength=NW)

    gsize = ((counts.max(0) + 127) // 128 * 128).astype(np.int64)   # [NW]
    kw = gsize // 128                                               # tiles/window

    # band-major tile schedule: band b = quantile quarter [b/4,(b+1)/4) of
    # every window, so run centers align across windows regardless of K_w
    NBANDS = 4
    kb = [[int(round(b * int(kw[w]) / NBANDS)) for b in range(NBANDS + 1)]
          for w in range(NW)]
    tile_list = []          # (w, k) in schedule order
    for b in range(NBANDS):
        for w in range(NW):
            for k in range(kb[w][b], kb[w][b + 1]):
                tile_list.append((w, k))
        # chunk-align each band (pad tiles trail inside the band's last chunk)
        while len(tile_list) % TPC != 0:
            tile_list.append((-1, -1))

    n_tiles = len(tile_list)
    total_slots = n_tiles * 128
    n_chunks = total_slots // CHUNK
    tile_w = np.array([w for w, _ in tile_list], np.int64)
    # slot base of each (w,k) tile
    tile_base = {}
    for t, (w, k) in enumerate(tile_list):
        if w >= 0:
            tile_base[(w, k)] = t * 128
    # first/last per (window, band) run
    tile_first = np.zeros(n_tiles, bool)
    tile_last = np.zeros(n_tiles, bool)
    tile_final = np.zeros(n_tiles, bool)
    for t, (w, k) in enumerate(tile_list):
        if w < 0:
            continue
        tile_first[t] = k in [kb[w][b] for b in range(NBANDS)]
        tile_last[t] = (k + 1) in [kb[w][b + 1] for b in range(NBANDS)]
        tile_final[t] = k + 1 == int(kw[w])

    # per-core slot arrays + per-tile block ranges
    idx_h = np.zeros((NCORES, 128, n_chunks * (CHUNK // 16)), np.int16)
    R_h = np.zeros((NCORES, 128, n_tiles * 128), NPF16)
    RT_h = np.zeros((NCORES, 128, n_tiles * 128), NPF16)
    pr_all = np.zeros((NCORES, total_slots), np.int64)
    off_all = np.full((NCORES, total_slots), -1, np.int64)
    tb_arr = np.full(NW * 32, -1, np.int64)
    for (w, k), sb in tile_base.items():
        tb_arr[w * 32 + k] = sb
    kidx = np.arange(128)
    for c in range(NCORES):
        s, d, w = per_core[c]
        grp_first = np.searchsorted(w, np.arange(NW))
        rank = np.arange(len(s)) - grp_first[w]
        q = (rank * gsize[w]) // counts[c][w]      # quantile-spread in window
        slot = tb_arr[w * 32 + (q // 128)] + (q % 128)
        assert (slot >= 0).all()
        pr_all[c][slot] = _perm_row(s)
        off_all[c][slot] = d & 127
        offs = off_all[c].reshape(n_tiles, 128)
        # R[p, t*128+k] = (off(slot t*128+p) == k); pads (off=-1) -> zero col
        R_h[c] = (offs[:, :, None] == kidx[None, None, :]) \
            .transpose(1, 0, 2).reshape(128, -1).astype(NPF16)
        # RT[p, t*128+e] = (off(slot t*128+e) == p)
        RT_h[c] = (offs[None, :, :] == kidx[:, None, None]) \
            .reshape(128, -1).astype(NPF16)

    # per-chunk gather calls with dynamic base (split if span > 31 blocks)
    real = off_all >= 0
    blk = np.where(real, pr_all // 1024, 1 << 30)
    blk_hi = np.where(real, pr_all // 1024, -1)
    gathers = []
    slot_base = np.zeros(total_slots, np.int64)
    for cidx in range(n_chunks):
        calls = []
        j = 0
        nlive = sum(1 for jj in range(TPC) if tile_w[cidx * TPC + jj] >= 0)
        while j < nlive:
            j0 = j
            s0 = cidx * CHUNK + j0 * 128
            lo = int(blk[:, s0:s0 + 128].min())
            hi = int(blk_hi[:, s0:s0 + 128].max())
            j += 1
            while j < nlive:
                s1 = cidx * CHUNK + j * 128
                nlo = min(lo, int(blk[:, s1:s1 + 128].min()))
                nhi = max(hi, int(blk_hi[:, s1:s1 + 128].max()))
                if nhi - nlo > 31:
                    break
                lo, hi = nlo, nhi
                j += 1
            if lo >= (1 << 30):
                lo = 0
            base = lo * 1024
            calls.append((j0, j - j0, int(base)))
            slot_base[cidx * CHUNK + j0 * 128: cidx * CHUNK + j * 128] = base
        if not calls:
            calls.append((0, TPC, 0))
        gathers.append(calls)

    for c in range(NCORES):
        gi = pr_all[c] - slot_base
        gi[~real[c]] = 0
        assert (gi >= 0).all() and (gi < 32768).all()
        gia = gi.reshape(n_chunks, CHUNK // 16, 16).transpose(0, 2, 1)
        idx_h[c] = np.tile(gia, (1, 8, 1)).transpose(1, 0, 2).reshape(128, -1)

    sched = dict(n_chunks=n_chunks, tile_w=tile_w.tolist(),
                 tile_first=tile_first.tolist(), tile_last=tile_last.tolist(),
                 tile_final=tile_final.tolist(), gathers=gathers)
    return sched, idx_h, R_h, RT_h


MARKS = []


def build(sched, debug=False, no_collective=False, reps=1):
    MARKS.clear()
    NW, NPAD, NBUCK, TBL_ROWS = _derived()
    n_chunks = sched["n_chunks"]
    tile_w = sched["tile_w"]
    tile_first = sched["tile_first"]
    tile_last = sched["tile_last"]
    tile_final = sched["tile_final"]
    gathers = sched["gathers"]
    n_tiles = n_chunks * TPC
    NT_DENSE = NPAD // 128
    NB_DENSE = (NT_DENSE + 7) // 8

    nc = bacc.Bacc(None, target_bir_lowering=False, num_swdge_queues=4)

    embT = nc.dram_tensor("embT", [D, NPAD], BF16, kind="ExternalInput")
    embTo = nc.dram_tensor("embTo", [D, OWN], BF16, kind="ExternalInput")
    # wNc = [Waux | Wad] so adw needs a single matmul per window
    w1c_in = nc.dram_tensor("w1c", [D, D + 2 * H1], BF16, kind="ExternalInput")
    w2c_in = nc.dram_tensor("w2c", [D, D + 2], BF16, kind="ExternalInput")
    b1t_in = nc.dram_tensor("b1t", [128, D], F32, kind="ExternalInput")
    b2t_in = nc.dram_tensor("b2t", [128, D], F32, kind="ExternalInput")
    ident_in = nc.dram_tensor("ident", [128, 128], F32, kind="ExternalInput")
    idx_in = nc.dram_tensor("idx16", [128, n_chunks * (CHUNK // 16)], I16, kind="ExternalInput")
    R_in = nc.dram_tensor("Rh", [128, n_tiles * 128], F16, kind="ExternalInput")
    RT_in = nc.dram_tensor("RTh", [128, n_tiles * 128], F16, kind="ExternalInput")
    out_own = nc.dram_tensor("out_own", [OWN, D], F32, kind="ExternalOutput")

    table = nc.dram_tensor("table", [TBL_ROWS, 128], F16)
    # window-quarter split of the activation exchange so the AllGather
    # pipelines behind sweep1's tail and dense2 starts on quarter 0
    QB = [0, 25, 50, 74, NW]
    ag_in = [nc.dram_tensor(f"ag_in{q}", [D, (QB[q + 1] - QB[q]) * 128], BF16)
             for q in range(4)]
    ag_out = [nc.dram_tensor(f"ag_out{q}", [NCORES * D, (QB[q + 1] - QB[q]) * 128],
                             BF16, addr_space="Shared")
              for q in range(4)]

    def q_of(w):
        for q in range(4):
            if w < QB[q + 1]:
                return q
        raise AssertionError

    def ag_in_slice(w0, w1):
        """Split window range [w0, w1) at quarter boundaries ->
        (tensor, col0, col1, out_off) pieces."""
        pieces = []
        w = w0
        while w < w1:
            q = q_of(w)
            we = min(w1, QB[q + 1])
            pieces.append((q, (w - QB[q]) * 128, (we - QB[q]) * 128,
                           (w - w0) * 128))
            w = we
        return pieces

    with tile.TileContext(nc) as tc:
        with tc.tile_pool(name="persist", bufs=1) as pp:
            b1t = pp.tile([128, D], F32)
            b2t = pp.tile([128, D], F32)
            ident = pp.tile([128, 128], F32)
            w1c = pp.tile([D, D + 2 * H1], BF16)
            w2c = pp.tile([D, D + 2], BF16)
            idx_s = pp.tile([128, n_chunks * (CHUNK // 16)], I16)
            adw = pp.tile([128, NW * H1], F16)
            adw2 = pp.tile([128, NW], F16)
            acc1 = pp.tile([128, NW * (D + H1)], F32)
            acc2 = pp.tile([128, NW * (D + 1)], F32)
            # broadcast constants: tensor_tensor + [128,1] const is far
            # cheaper than TENSOR_SCALAR with an immediate (~1.6us fixed)
            cNEG = pp.tile([128, 1], F32)
            cEPS = pp.tile([128, 1], F32)
            cNRM = pp.tile([128, 1], F32)
            nc.vector.memset(cNEG[:], NEG)
            nc.vector.memset(cEPS[:], 1e-16)
            nc.vector.memset(cNRM[:], 1e-12)
            for t_, s_ in [(b1t, b1t_in), (b2t, b2t_in), (ident, ident_in),
                           (w1c, w1c_in), (w2c, w2c_in),
                           (idx_s, idx_in)]:
                nc.sync.dma_start(out=t_[:], in_=s_[:])

            def fin1(w, fp, fpp):
                """Finalize window w of layer 1: softmax div, bias, ELU,
                transpose, write ag_in column block."""
                EC = D + H1
                den = fp.tile([128, H1], F32, tag="den")
                nc.vector.tensor_tensor(
                    out=den[:], in0=acc1[:, w * EC + D:(w + 1) * EC],
                    in1=cEPS[:].to_broadcast([128, H1]), op=mybir.AluOpType.add)
                rec = fp.tile([128, H1], F32, tag="rec")
                nc.vector.reciprocal(out=rec[:], in_=den[:])
                x2 = fp.tile([128, D], F32, tag="x2")
                nc.vector.tensor_tensor(
                    out=x2[:].rearrange("p (h k) -> p h k", k=C1),
                    in0=acc1[:, w * EC:w * EC + D].rearrange("p (h k) -> p h k", k=C1),
                    in1=rec[:, :, None].to_broadcast([128, H1, C1]),
                    op=mybir.AluOpType.mult)
                nc.vector.tensor_tensor(out=x2[:], in0=x2[:], in1=b1t[:],
                                        op=mybir.AluOpType.add)
                # elu(x) = relu(x) - relu(1 - exp(x))
                ex = fp.tile([128, D], F32, tag="ex")
                nc.scalar.activation(out=ex[:], in_=x2[:], func=ACT.Exp)
                u = fp.tile([128, D], F32, tag="u")
                nc.scalar.activation(out=u[:], in_=ex[:], func=ACT.Relu,
                                     scale=-1.0, bias=1.0)
                r = fp.tile([128, D], F32, tag="r")
                nc.scalar.activation(out=r[:], in_=x2[:], func=ACT.Relu)
                xe = fp.tile([128, D], F32, tag="xe")
                nc.vector.tensor_tensor(out=xe[:], in0=r[:], in1=u[:],
                                        op=mybir.AluOpType.subtract)
                pst = fpp.tile([D, 128], F32, tag="t")
                nc.tensor.transpose(out=pst[:], in_=xe[:], identity=ident[:])
                xt = fp.tile([D, 128], BF16, tag="xt")
                nc.scalar.activation(out=xt[:], in_=pst[:], func=ACT.Copy)
                q = q_of(w)
                c0 = (w - QB[q]) * 128
                nc.sync.dma_start(out=ag_in[q][:, c0:c0 + 128], in_=xt[:])

            def fin2(w, fp):
                """Finalize window w of layer 2: softmax div, bias, l2-norm,
                write out_own rows."""
                EC = D + 1
                den = fp.tile([128, 1], F32, tag="den")
                nc.vector.tensor_tensor(
                    out=den[:], in0=acc2[:, w * EC + D:(w + 1) * EC],
                    in1=cEPS[:], op=mybir.AluOpType.add)
                rec = fp.tile([128, 1], F32, tag="rec")
                nc.vector.reciprocal(out=rec[:], in_=den[:])
                o = fp.tile([128, D], F32, tag="o")
                nc.vector.tensor_tensor(
                    out=o[:], in0=acc2[:, w * EC:w * EC + D],
                    in1=rec[:].to_broadcast([128, D]), op=mybir.AluOpType.mult)
                nc.vector.tensor_tensor(out=o[:], in0=o[:], in1=b2t[:],
                                        op=mybir.AluOpType.add)
                sq = fp.tile([128, D], F32, tag="sq")
                ss = fp.tile([128, 1], F32, tag="ss")
                nc.scalar.activation(out=sq[:], in_=o[:], func=ACT.Square,
                                     accum_out=ss[:])
                nrm = fp.tile([128, 1], F32, tag="nr")
                nc.scalar.activation(out=nrm[:], in_=ss[:], func=ACT.Sqrt)
                nc.vector.tensor_tensor(out=nrm[:], in0=nrm[:], in1=cNRM[:],
                                        op=mybir.AluOpType.max)
                rn = fp.tile([128, 1], F32, tag="rn")
                nc.vector.reciprocal(out=rn[:], in_=nrm[:])
                of = fp.tile([128, D], F32, tag="of")
                nc.vector.tensor_tensor(out=of[:], in0=o[:],
                                        in1=rn[:].to_broadcast([128, D]),
                                        op=mybir.AluOpType.mult)
                nc.sync.dma_start(out=out_own[w * 128:(w + 1) * 128, :], in_=of[:])

            def layer_run(layer, rep):
                """One layer: adw (self-loops + per-window a_dst), dense table
                build, and the edge sweep — all under coexisting pools so the
                sweep's early bands overlap the tail of the dense build."""
                H = H1 if layer == 1 else 1
                CH = C1 if layer == 1 else D
                EC = D + H
                acc = acc1 if layer == 1 else acc2
                adwl = adw if layer == 1 else adw2
                dst_t = adwl
                wcomb = w1c if layer == 1 else w2c
                waux = wcomb[:, 0:EC]
                with tc.tile_pool(name=f"aw{layer}r{rep}", bufs=3) as ap, \
                     tc.tile_pool(name=f"dns{layer}r{rep}", bufs=3) as dp, \
                     tc.tile_pool(name=f"eg{layer}r{rep}", bufs=10) as gp, \
                     tc.tile_pool(name=f"er{layer}r{rep}", bufs=8) as rp, \
                     tc.tile_pool(name=f"em{layer}r{rep}", bufs=6) as mp, \
                     tc.tile_pool(name=f"fw{layer}r{rep}", bufs=3) as fwp, \
                     tc.tile_pool(name=f"epa{layer}r{rep}", bufs=2, space="PSUM") as pap, \
                     tc.tile_pool(name=f"fwp{layer}r{rep}", bufs=2, space="PSUM") as fpp, \
                     tc.tile_pool(name=f"epg{layer}r{rep}", bufs=2, space="PSUM") as pgp:

                    # ---- adw: per-window a_dst + self-loop contribution ----
                    app = [None]

                    def adw_step(w, ltb):
                        lt = ltb[:, (w % 8) * 128:(w % 8 + 1) * 128]
                        psh = app[0].tile([128, EC + H], F32, tag="h")
                        nc.tensor.matmul(out=psh[:], lhsT=lt, rhs=wcomb[:],
                                         start=True, stop=True)
                        nc.scalar.activation(out=dst_t[:, w * H:(w + 1) * H],
                                             in_=psh[:, EC:EC + H], func=ACT.Copy)
                        ho = ap.tile([128, EC], F16, tag="h16")
                        nc.scalar.activation(out=ho[:], in_=psh[:, 0:EC],
                                             func=ACT.Copy)
                        es = ap.tile([128, H], F32, tag="es")
                        nc.vector.tensor_tensor(out=es[:], in0=ho[:, D:D + H],
                                                in1=dst_t[:, w * H:(w + 1) * H],
                                                op=mybir.AluOpType.add)
                        lrs = ap.tile([128, H], F32, tag="lrs")
                        nc.vector.tensor_tensor(
                            out=lrs[:], in0=es[:],
                            in1=cNEG[:].to_broadcast([128, H]),
                            op=mybir.AluOpType.mult)
                        nc.vector.tensor_tensor(out=lrs[:], in0=lrs[:], in1=es[:],
                                                op=mybir.AluOpType.max)
                        wx = ap.tile([128, D], F16, tag="wx")
                        wx3 = wx[:].rearrange("p (h k) -> p h k", k=CH)
                        nc.scalar.activation(
                            out=wx3,
                            in_=lrs[:, :, None].to_broadcast([128, H, CH]),
                            func=ACT.Exp)
                        ms = ap.tile([128, D], F32, tag="ms")
                        nc.vector.tensor_tensor(out=ms[:], in0=ho[:, 0:D], in1=wx[:],
                                                op=mybir.AluOpType.mult)
                        nc.vector.tensor_tensor(
                            out=acc[:, w * EC:w * EC + D],
                            in0=acc[:, w * EC:w * EC + D], in1=ms[:],
                            op=mybir.AluOpType.add)
                        nc.vector.tensor_tensor(
                            out=acc[:, w * EC + D:(w + 1) * EC],
                            in0=acc[:, w * EC + D:(w + 1) * EC],
                            in1=wx3[:, :, 0],
                            op=mybir.AluOpType.add)

                    with tc.tile_pool(name=f"awp{layer}r{rep}", bufs=1,
                                      space="PSUM") as app_:
                        app[0] = app_
                        ltb = None
                        for w in range(NW):
                            if w % 8 == 0:
                                nwb = min(8, NW - w)
                                ltb = ap.tile([D, 8 * 128], BF16, tag="lb")
                                if layer == 1:
                                    nc.sync.dma_start(
                                        out=ltb[:, 0:nwb * 128],
                                        in_=embTo[:, w * 128:(w + nwb) * 128])
                                else:
                                    for (q, a0, a1, oo) in ag_in_slice(w, w + nwb):
                                        nc.sync.dma_start(
                                            out=ltb[:, oo:oo + (a1 - a0)],
                                            in_=ag_in[q][:, a0:a1])
                            adw_step(w, ltb)

                    # ---- dense: x @ Waux -> fp16 table rows ----
                    dpp = [None]

                    def dense_step(tb0):
                        nb = min(2, NB_DENSE - tb0)
                        lt = dp.tile([D, 2 * 1024], BF16, tag="lhs")
                        if layer == 1:
                            nc.sync.dma_start(
                                out=lt[:, 0:nb * 1024],
                                in_=embT[:, tb0 * 1024:(tb0 + nb) * 1024])
                        else:
                            # global tiles -> (core, window-quarter) runs
                            j = 0
                            while j < 8 * nb:
                                t = tb0 * 8 + j
                                co, wl = divmod(t, NW)
                                nrun = min(8 * nb - j, NW - wl)
                                for (q, a0, a1, oo) in ag_in_slice(wl, wl + nrun):
                                    nc.sync.dma_start(
                                        out=lt[:, j * 128 + oo:j * 128 + oo + (a1 - a0)],
                                        in_=ag_out[q][co * D:(co + 1) * D, a0:a1])
                                j += nrun
                        stg = dp.tile([128, 2 * 1024], F16, tag="stg")
                        for j in range(8 * nb):
                            ps = dpp[0].tile([128, EC], F32, tag="d")
                            nc.tensor.matmul(out=ps[:], lhsT=lt[:, j * 128:(j + 1) * 128],
                                             rhs=waux, start=True, stop=True)
                            if j % 2 == 0:
                                nc.scalar.activation(
                                    out=stg[:, j * 128:j * 128 + EC], in_=ps[:],
                                    func=ACT.Copy)
                            else:
                                nc.vector.tensor_copy(
                                    out=stg[:, j * 128:j * 128 + EC], in_=ps[:])
                        for b in range(nb):
                            nc.sync.dma_start(
                                out=table[(tb0 + b) * 1024:(tb0 + b + 1) * 1024]
                                .rearrange("(p j) k -> p (j k)", j=8),
                                in_=stg[:, b * 1024:(b + 1) * 1024])

                    with tc.tile_pool(name=f"dnp{layer}r{rep}", bufs=2,
                                      space="PSUM") as dpp_:
                        dpp[0] = dpp_
                        for tb0 in range(0, NB_DENSE, 2):
                            dense_step(tb0)

                    # ---- edge sweep ----
                    group_ps = {}
                    gq = 0
                    for c in range(n_chunks):
                        live = [j for j in range(TPC) if tile_w[c * TPC + j] >= 0]
                        assert live == list(range(len(live))), "pads must trail"
                        nl = len(live)
                        # one-hot streams from host (R: edge-major, RT: node-major);
                        # emitted before the gather so they prefetch ahead
                        if live:
                            Rt = rp.tile([128, TPC * 128], F16, tag="R")
                            nc.sync.dma_start(
                                out=Rt[:, 0:nl * 128],
                                in_=R_in[:, (c * TPC) * 128:(c * TPC + nl) * 128])
                            RTt = rp.tile([128, TPC * 128], F16, tag="RT")
                            nc.sync.dma_start(
                                out=RTt[:, 0:nl * 128],
                                in_=RT_in[:, (c * TPC) * 128:(c * TPC + nl) * 128])
                        ght = gp.tile([128, TPC * 128], F16, tag="ght")
                        ght3g = ght[:].rearrange("p (a k) -> p a k", k=128)
                        for (j0, ntl, base) in gathers[c]:
                            hi = min(base + 32768, TBL_ROWS)
                            cb = c * (CHUNK // 16)
                            nc.gpsimd.dma_gather(
                                ght3g[:, j0:j0 + ntl, :],
                                table[base:hi, :],
                                idx_s[:, cb + j0 * 8:cb + (j0 + ntl) * 8],
                                ntl * 128, ntl * 128, 128, elem_step=128,
                                queue_num=gq % 4)
                            gq += 1
                        if not live:
                            continue
                        ght3 = ght[:].rearrange("p (a k) -> p a k", k=128)
                        # per-edge a_dst via PE gather: psa = RT_j^T @ adw_w
                        psa = pap.tile([128, nl * H], F32, tag="a", name=f"psa{c}")
                        for j in live:
                            w = tile_w[c * TPC + j]
                            nc.tensor.matmul(
                                out=psa[:, j * H:(j + 1) * H],
                                lhsT=RTt[:, j * 128:(j + 1) * 128],
                                rhs=adwl[:, w * H:(w + 1) * H],
                                start=True, stop=True)
                        ew = mp.tile([128, nl * H], F32, tag="ew", name=f"ew{c}")
                        nc.vector.tensor_tensor(
                            out=ew[:].rearrange("p (a h) -> p a h", h=H),
                            in0=psa[:].rearrange("p (a h) -> p a h", h=H),
                            in1=ght3[:, 0:nl, D:D + H],
                            op=mybir.AluOpType.add)
                        lr = mp.tile([128, nl * H], F32, tag="lr", name=f"lr{c}")
                        nc.vector.tensor_tensor(
                            out=lr[:], in0=ew[:],
                            in1=cNEG[:].to_broadcast([128, nl * H]),
                            op=mybir.AluOpType.mult)
                        nc.vector.tensor_tensor(out=lr[:], in0=lr[:], in1=ew[:],
                                                op=mybir.AluOpType.max)
                        # exp, pre-expanded across the C dim (Act) -> DVE mult
                        we16 = mp.tile([128, nl * D], F16, tag="we", name=f"we{c}")
                        we4 = we16[:].rearrange("p (a h k) -> p a h k", h=H, k=CH)
                        nc.scalar.activation(
                            out=we4,
                            in_=lr[:].rearrange("p (a h) -> p a h", h=H)[:, :, :, None]
                                .to_broadcast([128, nl, H, CH]),
                            func=ACT.Exp)
                        msgt = mp.tile([128, nl * EC], F16, tag="msg", name=f"msg{c}")
                        msgt3 = msgt[:].rearrange("p (a k) -> p a k", k=EC)
                        # denominator cols written directly by a second exp (Act)
                        nc.scalar.activation(
                            out=msgt3[:, :, D:D + H],
                            in_=lr[:].rearrange("p (a h) -> p a h", h=H),
                            func=ACT.Exp)
                        nc.vector.tensor_tensor(
                            out=msgt3[:, :, 0:D].rearrange("p a (h k) -> p a h k", k=CH),
                            in0=ght3[:, 0:nl, 0:D].rearrange("p a (h k) -> p a h k", k=CH),
                            in1=we4,
                            op=mybir.AluOpType.mult)
                        for j in live:
                            t = c * TPC + j
                            w = tile_w[t]
                            if tile_first[t]:
                                group_ps[w] = pgp.tile([128, EC], F32, tag="g", name=f"grp{w}")
                            ps = group_ps[w]
                            nc.tensor.matmul(
                                out=ps[:], lhsT=Rt[:, j * 128:(j + 1) * 128],
                                rhs=msgt[:, j * EC:(j + 1) * EC],
                                start=tile_first[t], stop=tile_last[t])
                            if tile_last[t]:
                                nc.vector.tensor_tensor(
                                    out=acc[:, w * EC:(w + 1) * EC],
                                    in0=acc[:, w * EC:(w + 1) * EC],
                                    in1=ps[:], op=mybir.AluOpType.add)
                                del group_ps[w]
                                if tile_final[t]:
                                    if layer == 1:
                                        fin1(w, fwp, fpp)
                                    else:
                                        fin2(w, fwp)
                    assert not group_ps

            for rep in range(reps):
                nc.vector.memset(acc1[:], 0.0)
                nc.vector.memset(acc2[:], 0.0)
                # ================= layer 1 =================
                if rep == 0:
                    MARKS.append(("layer1", nc.next_id()))
                layer_run(1, rep)
                if rep == 0:
                    MARKS.append(("collective", nc.next_id()))
                if no_collective:
                    with tc.tile_pool(name=f"agcr{rep}", bufs=2) as acp:
                        for cc in range(NCORES):
                            for q in range(4):
                                t_ = acp.tile([D, 25 * 128], BF16, tag="agc")
                                w_ = (QB[q + 1] - QB[q]) * 128
                                nc.sync.dma_start(out=t_[:, 0:w_], in_=ag_in[q][:])
                                nc.sync.dma_start(
                                    out=ag_out[q][cc * D:(cc + 1) * D, :],
                                    in_=t_[:, 0:w_])
                else:
                    for q in range(4):
                        nc.gpsimd.collective_compute(
                            "AllGather", mybir.AluOpType.bypass,
                            ins=[ag_in[q][:]], outs=[ag_out[q][:]],
                            replica_groups=[list(range(NCORES))])

                # ================= layer 2 =================
                if rep == 0:
                    MARKS.append(("layer2", nc.next_id()))
                layer_run(2, rep)
    return nc


def make_inputs(edge_index, emb, W1, a_src1, a_dst1, b1, W2, a_src2, a_dst2, b2):
    NW, NPAD, NBUCK, TBL_ROWS = _derived()
    sched, idx_h, R_h, RT_h = prep(edge_index)

    W1 = np.asarray(W1, np.float32)
    a_s1 = np.asarray(a_src1, np.float32)
    a_d1 = np.asarray(a_dst1, np.float32)
    As = np.zeros((D, H1), np.float32)
    Ad = np.zeros((D, H1), np.float32)
    for h in range(H1):
        As[h * C1:(h + 1) * C1, h] = a_s1[h]
        Ad[h * C1:(h + 1) * C1, h] = a_d1[h]
    # [Waux | Wad] = [W | W@As | W@Ad]
    w1c = np.concatenate([W1, W1 @ As, W1 @ Ad], 1).astype(NPBF16)
    W2 = np.asarray(W2, np.float32)
    w2c = np.concatenate([W2, W2 @ np.asarray(a_src2, np.float32).T,
                          W2 @ np.asarray(a_dst2, np.float32).T], 1).astype(NPBF16)

    embT = np.zeros((D, NPAD), NPBF16)
    embT[:, :N] = np.asarray(emb, np.float32).T.astype(NPBF16)
    ident = np.eye(128, dtype=np.float32)
    b1t = np.broadcast_to(np.asarray(b1, np.float32)[None, :], (128, D)).copy()
    b2t = np.broadcast_to(np.asarray(b2, np.float32)[None, :], (128, D)).copy()

    in_maps = []
    for c in range(NCORES):
        in_maps.append({
            "embT": embT, "embTo": np.ascontiguousarray(embT[:, c * OWN:(c + 1) * OWN]),
            "w1c": w1c, "w2c": w2c,
            "b1t": b1t, "b2t": b2t, "ident": ident,
            "idx16": idx_h[c], "Rh": R_h[c], "RTh": RT_h[c],
        })
    return sched, in_maps


def kernel(edge_index, emb, W1, a_src1, a_dst1, b1, W2, a_src2, a_dst2, b2):
    sched, in_maps = make_inputs(edge_index, emb, W1, a_src1, a_dst1, b1,
                                 W2, a_src2, a_dst2, b2)
    nc = build(sched)
    nc.finalize()
    res = run_bass_kernel_spmd(nc, in_maps, core_ids=list(range(NCORES)))
    out = np.zeros((N, D), np.float32)
    for c in range(NCORES):
        lo, hi = c * OWN, min((c + 1) * OWN, N)
        if lo < N:
            out[lo:hi] = res.results[c]["out_own"][:hi - lo]
    return out


# revision 27
# speedup vs baseline: 1.1590x; 1.1590x over previous
"""Bass/Trainium2 kernel for the 2-layer GAT (nn_GAT_11106785427688).

Strategy (8 NeuronCores, SPMD single NEFF):
- dst-ownership sharding: core c owns nodes [c*OWN, (c+1)*OWN); it receives
  every edge whose dst it owns (~137K edges), so segment-softmax denominators
  and message sums complete locally -- no all-reduce. One AllGather of the
  layer-1 activations between layers; host assembles the final output from
  per-core slices.
- Per-edge gather of packed [h | a_src.h] rows (fp16, 256B) from an HBM table
  via the SWDGE dma_gather custom op (int16 indices -> src buckets of 32768
  rows; table rows permuted so the dense phase writes 2KB-contiguous runs).
- No indexed scatter (HW dma_scatter_add loses duplicate updates): edges are
  grouped by 128-node dst window; the one-hot R [edges x nodes] and its
  transpose RT [nodes x edges] are PRECOMPUTED ON HOST (pure edge-index
  preprocessing) and streamed from HBM, so the DVE never builds one-hots.
  R turns segment-sum into PE matmul accumulated in PSUM; RT gathers the
  per-window a_dst values to edges via PE. Softmax division is deferred:
  out = (sum_e w*h[src]) / (sum_e w).
- exp(leakyrelu(e)) computed without max-subtraction (shift-invariant).
- adw_fill (self-loops + per-window a_dst) is interleaved with the dense
  table build so PE/Act/DVE/DMA overlap instead of running serial phases.
"""
import numpy as np
import ml_dtypes

from concourse import bacc, mybir
import concourse.tile as tile
from concourse.bass_utils import run_bass_kernel_spmd

# ---- problem constants ----
N = 100000
D = 64
H1, C1 = 4, 16
NEG = 0.2
NCORES = 8
OWN = 12544                 # 98 windows * 128 per core
BUCK = 32768
CHUNK = 1024                # gather idxs per dma_gather call (ring limit)
TPC = CHUNK // 128          # tiles per chunk = 8

F16 = mybir.dt.float16
F32 = mybir.dt.float32
BF16 = mybir.dt.bfloat16
I16 = mybir.dt.int16
NPF16 = np.float16
NPBF16 = ml_dtypes.bfloat16

ACT = mybir.ActivationFunctionType


def _derived():
    NW = OWN // 128
    NPAD = NCORES * OWN
    NBUCK = (NPAD + BUCK - 1) // BUCK
    TBL_ROWS = NBUCK * BUCK
    return NW, NPAD, NBUCK, TBL_ROWS


def _perm_row(src):
    """Permuted table row for node src: tb*1024 + p*8 + j (write-friendly)."""
    tb, r = np.divmod(src, 1024)
    j, p = np.divmod(r, 128)
    return tb * 1024 + p * 8 + j


def prep(edge_index):
    """Vectorized host prep: quantile-banded schedule.

    Per-(core,window) edges sorted by src, quantile-spread into the padded
    window group (G_w = roundup128(max-over-cores)). Window tiles are split
    into bands of <=3 tiles; the schedule is band-major so consecutive tiles
    cover the same src-quantile region. Each 1024-slot chunk then spans <=~31
    perm-blocks and gets ONE dma_gather call with a dynamic host-computed
    base (int16 idx). Bands are chunk-aligned (pad tiles trail per band).

    Also builds, per core, the fp16 one-hot streams R (edge-major: used as
    matmul lhsT for the per-window segment sums) and RT (node-major: used as
    lhsT to gather per-window a_dst values to edge positions).
    """
    NW, NPAD, NBUCK, TBL_ROWS = _derived()
    # self-loops are handled densely in adw_fill, not in the gather sweep
    src = np.asarray(edge_index[0])
    dst = np.asarray(edge_index[1])
    owner = dst // OWN

    per_core = []
    counts = np.zeros((NCORES, NW), np.int64)
    for c in range(NCORES):
        m = owner == c
        s = src[m]
        d = dst[m] - c * OWN
        w = d >> 7
        order = np.lexsort((s, w))
        s, d, w = s[order], d[order], w[order]
        per_core.append((s, d, w))
        counts[c] = np.bincount(w, minlength=NW)

    gsize = ((counts.max(0) + 127) // 128 * 128).astype(np.int64)   # [NW]
    kw = gsize // 128                                               # tiles/window

    # band-major tile schedule: band b = quantile quarter [b/4,(b+1)/4) of
    # every window, so run centers align across windows regardless of K_w
    NBANDS = 4
    kb = [[int(round(b * int(kw[w]) / NBANDS)) for b in range(NBANDS + 1)]
          for w in range(NW)]
    tile_list = []          # (w, k) in schedule order
    for b in range(NBANDS):
        for w in range(NW):
            for k in range(kb[w][b], kb[w][b + 1]):
                tile_list.append((w, k))
        # chunk-align each band (pad tiles trail inside the band's last chunk)
        while len(tile_list) % TPC != 0:
            tile_list.append((-1, -1))

    n_tiles = len(tile_list)
    total_slots = n_tiles * 128
    n_chunks = total_slots // CHUNK
    tile_w = np.array([w for w, _ in tile_list], np.int64)
    # slot base of each (w,k) tile
    tile_base = {}
    for t, (w, k) in enumerate(tile_list):
        if w >= 0:
            tile_base[(w, k)] = t * 128
    # first/last per (window, band) run
    tile_first = np.zeros(n_tiles, bool)
    tile_last = np.zeros(n_tiles, bool)
    tile_final = np.zeros(n_tiles, bool)
    for t, (w, k) in enumerate(tile_list):
        if w < 0:
            continue
        tile_first[t] = k in [kb[w][b] for b in range(NBANDS)]
        tile_last[t] = (k + 1) in [kb[w][b + 1] for b in range(NBANDS)]
        tile_final[t] = k + 1 == int(kw[w])

    # per-core slot arrays + per-tile block ranges
    idx_h = np.zeros((NCORES, 128, n_chunks * (CHUNK // 16)), np.int16)
    R_h = np.zeros((NCORES, 128, n_tiles * 128), NPF16)
    RT_h = np.zeros((NCORES, 128, n_tiles * 128), NPF16)
    pr_all = np.zeros((NCORES, total_slots), np.int64)
    off_all = np.full((NCORES, total_slots), -1, np.int64)
    tb_arr = np.full(NW * 32, -1, np.int64)
    for (w, k), sb in tile_base.items():
        tb_arr[w * 32 + k] = sb
    kidx = np.arange(128)
    for c in range(NCORES):
        s, d, w = per_core[c]
        grp_first = np.searchsorted(w, np.arange(NW))
        rank = np.arange(len(s)) - grp_first[w]
        q = (rank * gsize[w]) // counts[c][w]      # quantile-spread in window
        slot = tb_arr[w * 32 + (q // 128)] + (q % 128)
        assert (slot >= 0).all()
        pr_all[c][slot] = _perm_row(s)
        off_all[c][slot] = d & 127
        offs = off_all[c].reshape(n_tiles, 128)
        # R[p, t*128+k] = (off(slot t*128+p) == k); pads (off=-1) -> zero col
        R_h[c] = (offs[:, :, None] == kidx[None, None, :]) \
            .transpose(1, 0, 2).reshape(128, -1).astype(NPF16)
        # RT[p, t*128+e] = (off(slot t*128+e) == p)
        RT_h[c] = (offs[None, :, :] == kidx[:, None, None]) \
            .reshape(128, -1).astype(NPF16)

    # per-chunk gather calls with dynamic base (split if span > 31 blocks)
    real = off_all >= 0
    blk = np.where(real, pr_all // 1024, 1 << 30)
    blk_hi = np.where(real, pr_all // 1024, -1)
    gathers = []
    slot_base = np.zeros(total_slots, np.int64)
    for cidx in range(n_chunks):
        calls = []
        j = 0
        nlive = sum(1 for jj in range(TPC) if tile_w[cidx * TPC + jj] >= 0)
        while j < nlive:
            j0 = j
            s0 = cidx * CHUNK + j0 * 128
            lo = int(blk[:, s0:s0 + 128].min())
            hi = int(blk_hi[:, s0:s0 + 128].max())
            j += 1
            while j < nlive:
                s1 = cidx * CHUNK + j * 128
                nlo = min(lo, int(blk[:, s1:s1 + 128].min()))
                nhi = max(hi, int(blk_hi[:, s1:s1 + 128].max()))
                if nhi - nlo > 31:
                    break
                lo, hi = nlo, nhi
                j += 1
            if lo >= (1 << 30):
                lo = 0
                hi = 0
            base = lo * 1024
            hi_row = min((hi + 1) * 1024, base + 32768)
            calls.append((j0, j - j0, int(base), int(hi_row)))
            slot_base[cidx * CHUNK + j0 * 128: cidx * CHUNK + j * 128] = base
        if not calls:
            calls.append((0, TPC, 0, 1024))
        gathers.append(calls)

    for c in range(NCORES):
        gi = pr_all[c] - slot_base
        gi[~real[c]] = 0
        assert (gi >= 0).all() and (gi < 32768).all()
        gia = gi.reshape(n_chunks, CHUNK // 16, 16).transpose(0, 2, 1)
        idx_h[c] = np.tile(gia, (1, 8, 1)).transpose(1, 0, 2).reshape(128, -1)

    sched = dict(n_chunks=n_chunks, tile_w=tile_w.tolist(),
                 tile_first=tile_first.tolist(), tile_last=tile_last.tolist(),
                 tile_final=tile_final.tolist(), gathers=gathers)
    return sched, idx_h, R_h, RT_h


MARKS = []


def build(sched, debug=False, no_collective=False, reps=1):
    MARKS.clear()
    NW, NPAD, NBUCK, TBL_ROWS = _derived()
    n_chunks = sched["n_chunks"]
    tile_w = sched["tile_w"]
    tile_first = sched["tile_first"]
    tile_last = sched["tile_last"]
    tile_final = sched["tile_final"]
    gathers = sched["gathers"]
    n_tiles = n_chunks * TPC
    NT_DENSE = NPAD // 128
    NB_DENSE = (NT_DENSE + 7) // 8

    nc = bacc.Bacc(None, target_bir_lowering=False, num_swdge_queues=4)

    embT = nc.dram_tensor("embT", [D, NPAD], BF16, kind="ExternalInput")
    embTo = nc.dram_tensor("embTo", [D, OWN], BF16, kind="ExternalInput")
    # wNc = [Waux | Wad] so adw needs a single matmul per window
    w1c_in = nc.dram_tensor("w1c", [D, D + 2 * H1], BF16, kind="ExternalInput")
    w2c_in = nc.dram_tensor("w2c", [D, D + 2], BF16, kind="ExternalInput")
    b1t_in = nc.dram_tensor("b1t", [128, D], F32, kind="ExternalInput")
    b2t_in = nc.dram_tensor("b2t", [128, D], F32, kind="ExternalInput")
    ident_in = nc.dram_tensor("ident", [128, 128], F32, kind="ExternalInput")
    idx_in = nc.dram_tensor("idx16", [128, n_chunks * (CHUNK // 16)], I16, kind="ExternalInput")
    R_in = nc.dram_tensor("Rh", [128, n_tiles * 128], F16, kind="ExternalInput")
    RT_in = nc.dram_tensor("RTh", [128, n_tiles * 128], F16, kind="ExternalInput")
    out_own = nc.dram_tensor("out_own", [OWN, D], F32, kind="ExternalOutput")

    table = nc.dram_tensor("table", [TBL_ROWS, 128], F16)
    # window-quarter split of the activation exchange so the AllGather
    # pipelines behind sweep1's tail and dense2 starts on quarter 0
    QB = [0, 25, 50, 74, NW]
    ag_in = [nc.dram_tensor(f"ag_in{q}", [D, (QB[q + 1] - QB[q]) * 128], BF16)
             for q in range(4)]
    ag_out = [nc.dram_tensor(f"ag_out{q}", [NCORES * D, (QB[q + 1] - QB[q]) * 128],
                             BF16, addr_space="Shared")
              for q in range(4)]

    def q_of(w):
        for q in range(4):
            if w < QB[q + 1]:
                return q
        raise AssertionError

    def ag_in_slice(w0, w1):
        """Split window range [w0, w1) at quarter boundaries ->
        (tensor, col0, col1, out_off) pieces."""
        pieces = []
        w = w0
        while w < w1:
            q = q_of(w)
            we = min(w1, QB[q + 1])
            pieces.append((q, (w - QB[q]) * 128, (we - QB[q]) * 128,
                           (w - w0) * 128))
            w = we
        return pieces

    with tile.TileContext(nc) as tc:
        with tc.tile_pool(name="persist", bufs=1) as pp:
            b1t = pp.tile([128, D], F32)
            b2t = pp.tile([128, D], F32)
            ident = pp.tile([128, 128], F32)
            w1c = pp.tile([D, D + 2 * H1], BF16)
            w2c = pp.tile([D, D + 2], BF16)
            idx_s = pp.tile([128, n_chunks * (CHUNK // 16)], I16)
            adw = pp.tile([128, NW * H1], F16)
            adw2 = pp.tile([128, NW], F16)
            acc1 = pp.tile([128, NW * (D + H1)], F32)
            acc2 = pp.tile([128, NW * (D + 1)], F32)
            # broadcast constants: tensor_tensor + [128,1] const is far
            # cheaper than TENSOR_SCALAR with an immediate (~1.6us fixed)
            cNEG = pp.tile([128, 1], F32)
            cEPS = pp.tile([128, 1], F32)
            cNRM = pp.tile([128, 1], F32)
            nc.vector.memset(cNEG[:], NEG)
            nc.vector.memset(cEPS[:], 1e-16)
            nc.vector.memset(cNRM[:], 1e-12)
            for t_, s_ in [(b1t, b1t_in), (b2t, b2t_in), (ident, ident_in),
                           (w1c, w1c_in), (w2c, w2c_in),
                           (idx_s, idx_in)]:
                nc.sync.dma_start(out=t_[:], in_=s_[:])

            def fin1(w, fp, fpp):
                """Finalize window w of layer 1: softmax div, bias, ELU,
                transpose, write ag_in column block."""
                EC = D + H1
                den = fp.tile([128, H1], F32, tag="den")
                nc.vector.tensor_tensor(
                    out=den[:], in0=acc1[:, w * EC + D:(w + 1) * EC],
                    in1=cEPS[:].to_broadcast([128, H1]), op=mybir.AluOpType.add)
                rec = fp.tile([128, H1], F32, tag="rec")
                nc.vector.reciprocal(out=rec[:], in_=den[:])
                x2 = fp.tile([128, D], F32, tag="x2")
                nc.vector.tensor_tensor(
                    out=x2[:].rearrange("p (h k) -> p h k", k=C1),
                    in0=acc1[:, w * EC:w * EC + D].rearrange("p (h k) -> p h k", k=C1),
                    in1=rec[:, :, None].to_broadcast([128, H1, C1]),
                    op=mybir.AluOpType.mult)
                nc.vector.tensor_tensor(out=x2[:], in0=x2[:], in1=b1t[:],
                                        op=mybir.AluOpType.add)
                # elu(x) = relu(x) - relu(1 - exp(x))
                ex = fp.tile([128, D], F32, tag="ex")
                nc.scalar.activation(out=ex[:], in_=x2[:], func=ACT.Exp)
                u = fp.tile([128, D], F32, tag="u")
                nc.scalar.activation(out=u[:], in_=ex[:], func=ACT.Relu,
                                     scale=-1.0, bias=1.0)
                r = fp.tile([128, D], F32, tag="r")
                nc.scalar.activation(out=r[:], in_=x2[:], func=ACT.Relu)
                xe = fp.tile([128, D], F32, tag="xe")
                nc.vector.tensor_tensor(out=xe[:], in0=r[:], in1=u[:],
                                        op=mybir.AluOpType.subtract)
                pst = fpp.tile([D, 128], F32, tag="t")
                nc.tensor.transpose(out=pst[:], in_=xe[:], identity=ident[:])
                xt = fp.tile([D, 128], BF16, tag="xt")
                nc.scalar.activation(out=xt[:], in_=pst[:], func=ACT.Copy)
                q = q_of(w)
                c0 = (w - QB[q]) * 128
                nc.sync.dma_start(out=ag_in[q][:, c0:c0 + 128], in_=xt[:])

            def fin2(w, fp):
                """Finalize window w of layer 2: softmax div, bias, l2-norm,
                write out_own rows."""
                EC = D + 1
                den = fp.tile([128, 1], F32, tag="den")
                nc.vector.tensor_tensor(
                    out=den[:], in0=acc2[:, w * EC + D:(w + 1) * EC],
                    in1=cEPS[:], op=mybir.AluOpType.add)
                rec = fp.tile([128, 1], F32, tag="rec")
                nc.vector.reciprocal(out=rec[:], in_=den[:])
                o = fp.tile([128, D], F32, tag="o")
                nc.vector.tensor_tensor(
                    out=o[:], in0=acc2[:, w * EC:w * EC + D],
                    in1=rec[:].to_broadcast([128, D]), op=mybir.AluOpType.mult)
                nc.vector.tensor_tensor(out=o[:], in0=o[:], in1=b2t[:],
                                        op=mybir.AluOpType.add)
                sq = fp.tile([128, D], F32, tag="sq")
                ss = fp.tile([128, 1], F32, tag="ss")
                nc.scalar.activation(out=sq[:], in_=o[:], func=ACT.Square,
                                     accum_out=ss[:])
                nrm = fp.tile([128, 1], F32, tag="nr")
                nc.scalar.activation(out=nrm[:], in_=ss[:], func=ACT.Sqrt)
                nc.vector.tensor_tensor(out=nrm[:], in0=nrm[:], in1=cNRM[:],
                                        op=mybir.AluOpType.max)
                rn = fp.tile([128, 1], F32, tag="rn")
                nc.vector.reciprocal(out=rn[:], in_=nrm[:])
                of = fp.tile([128, D], F32, tag="of")
                nc.vector.tensor_tensor(out=of[:], in0=o[:],
                                        in1=rn[:].to_broadcast([128, D]),
                                        op=mybir.AluOpType.mult)
                nc.sync.dma_start(out=out_own[w * 128:(w + 1) * 128, :], in_=of[:])

            def layer_run(layer, rep):
                """One layer: adw (self-loops + per-window a_dst), dense table
                build, and the edge sweep — all under coexisting pools so the
                sweep's early bands overlap the tail of the dense build."""
                H = H1 if layer == 1 else 1
                CH = C1 if layer == 1 else D
                EC = D + H
                acc = acc1 if layer == 1 else acc2
                adwl = adw if layer == 1 else adw2
                dst_t = adwl
                wcomb = w1c if layer == 1 else w2c
                waux = wcomb[:, 0:EC]
                with tc.tile_pool(name=f"aw{layer}r{rep}", bufs=3) as ap, \
                     tc.tile_pool(name=f"dns{layer}r{rep}", bufs=3) as dp, \
                     tc.tile_pool(name=f"eg{layer}r{rep}", bufs=10) as gp, \
                     tc.tile_pool(name=f"er{layer}r{rep}", bufs=8) as rp, \
                     tc.tile_pool(name=f"em{layer}r{rep}", bufs=6) as mp, \
                     tc.tile_pool(name=f"fw{layer}r{rep}", bufs=3) as fwp, \
                     tc.tile_pool(name=f"epa{layer}r{rep}", bufs=2, space="PSUM") as pap, \
                     tc.tile_pool(name=f"fwp{layer}r{rep}", bufs=2, space="PSUM") as fpp, \
                     tc.tile_pool(name=f"epg{layer}r{rep}", bufs=2, space="PSUM") as pgp:

                    # ---- adw: per-window a_dst + self-loop contribution ----
                    app = [None]

                    def adw_step(w, ltb):
                        lt = ltb[:, (w % 8) * 128:(w % 8 + 1) * 128]
                        psh = app[0].tile([128, EC + H], F32, tag="h")
                        nc.tensor.matmul(out=psh[:], lhsT=lt, rhs=wcomb[:],
                                         start=True, stop=True)
                        nc.scalar.activation(out=dst_t[:, w * H:(w + 1) * H],
                                             in_=psh[:, EC:EC + H], func=ACT.Copy)
                        ho = ap.tile([128, EC], F16, tag="h16")
                        nc.scalar.activation(out=ho[:], in_=psh[:, 0:EC],
                                             func=ACT.Copy)
                        es = ap.tile([128, H], F32, tag="es")
                        nc.vector.tensor_tensor(out=es[:], in0=ho[:, D:D + H],
                                                in1=dst_t[:, w * H:(w + 1) * H],
                                                op=mybir.AluOpType.add)
                        lrs = ap.tile([128, H], F32, tag="lrs")
                        nc.vector.tensor_tensor(
                            out=lrs[:], in0=es[:],
                            in1=cNEG[:].to_broadcast([128, H]),
                            op=mybir.AluOpType.mult)
                        nc.vector.tensor_tensor(out=lrs[:], in0=lrs[:], in1=es[:],
                                                op=mybir.AluOpType.max)
                        wx = ap.tile([128, D], F16, tag="wx")
                        wx3 = wx[:].rearrange("p (h k) -> p h k", k=CH)
                        nc.scalar.activation(
                            out=wx3,
                            in_=lrs[:, :, None].to_broadcast([128, H, CH]),
                            func=ACT.Exp)
                        ms = ap.tile([128, D], F32, tag="ms")
                        nc.vector.tensor_tensor(out=ms[:], in0=ho[:, 0:D], in1=wx[:],
                                                op=mybir.AluOpType.mult)
                        nc.vector.tensor_tensor(
                            out=acc[:, w * EC:w * EC + D],
                            in0=acc[:, w * EC:w * EC + D], in1=ms[:],
                            op=mybir.AluOpType.add)
                        nc.vector.tensor_tensor(
                            out=acc[:, w * EC + D:(w + 1) * EC],
                            in0=acc[:, w * EC + D:(w + 1) * EC],
                            in1=wx3[:, :, 0],
                            op=mybir.AluOpType.add)

                    # ---- dense: x @ Waux -> fp16 table rows ----
                    dpp = [None]

                    def dense_step(tb0):
                        nb = min(2, NB_DENSE - tb0)
                        lt = dp.tile([D, 2 * 1024], BF16, tag="lhs")
                        if layer == 1:
                            nc.sync.dma_start(
                                out=lt[:, 0:nb * 1024],
                                in_=embT[:, tb0 * 1024:(tb0 + nb) * 1024])
                        else:
                            # global tiles -> (core, window-quarter) runs
                            j = 0
                            while j < 8 * nb:
                                t = tb0 * 8 + j
                                co, wl = divmod(t, NW)
                                nrun = min(8 * nb - j, NW - wl)
                                for (q, a0, a1, oo) in ag_in_slice(wl, wl + nrun):
                                    nc.sync.dma_start(
                                        out=lt[:, j * 128 + oo:j * 128 + oo + (a1 - a0)],
                                        in_=ag_out[q][co * D:(co + 1) * D, a0:a1])
                                j += nrun
                        stg = dp.tile([128, 2 * 1024], F16, tag="stg")
                        for j in range(8 * nb):
                            ps = dpp[0].tile([128, EC], F32, tag="d")
                            nc.tensor.matmul(out=ps[:], lhsT=lt[:, j * 128:(j + 1) * 128],
                                             rhs=waux, start=True, stop=True)
                            if j % 2 == 0:
                                nc.scalar.activation(
                                    out=stg[:, j * 128:j * 128 + EC], in_=ps[:],
                                    func=ACT.Copy)
                            else:
                                nc.vector.tensor_copy(
                                    out=stg[:, j * 128:j * 128 + EC], in_=ps[:])
                        for b in range(nb):
                            nc.sync.dma_start(
                                out=table[(tb0 + b) * 1024:(tb0 + b + 1) * 1024]
                                .rearrange("(p j) k -> p (j k)", j=8),
                                in_=stg[:, b * 1024:(b + 1) * 1024])

                    # dense first: the sweep's early bands wait on the table,
                    # so table blocks get scheduling priority over adw
                    with tc.tile_pool(name=f"dnp{layer}r{rep}", bufs=2,
                                      space="PSUM") as dpp_:
                        dpp[0] = dpp_
                        for tb0 in range(0, NB_DENSE, 2):
                            dense_step(tb0)

                    with tc.tile_pool(name=f"awp{layer}r{rep}", bufs=1,
                                      space="PSUM") as app_:
                        app[0] = app_
                        ltb = None
                        for w in range(NW):
                            if w % 8 == 0:
                                nwb = min(8, NW - w)
                                ltb = ap.tile([D, 8 * 128], BF16, tag="lb")
                                if layer == 1:
                                    nc.sync.dma_start(
                                        out=ltb[:, 0:nwb * 128],
                                        in_=embTo[:, w * 128:(w + nwb) * 128])
                                else:
                                    for (q, a0, a1, oo) in ag_in_slice(w, w + nwb):
                                        nc.sync.dma_start(
                                            out=ltb[:, oo:oo + (a1 - a0)],
                                            in_=ag_in[q][:, a0:a1])
                            adw_step(w, ltb)

                    # ---- edge sweep ----
                    group_ps = {}
                    gq = 0
                    for c in range(n_chunks):
                        live = [j for j in range(TPC) if tile_w[c * TPC + j] >= 0]
                        assert live == list(range(len(live))), "pads must trail"
                        nl = len(live)
                        # one-hot streams from host (R: edge-major, RT: node-major);
                        # emitted before the gather so they prefetch ahead
                        if live:
                            Rt = rp.tile([128, TPC * 128], F16, tag="R")
                            nc.sync.dma_start(
                                out=Rt[:, 0:nl * 128],
                                in_=R_in[:, (c * TPC) * 128:(c * TPC + nl) * 128])
                            RTt = rp.tile([128, TPC * 128], F16, tag="RT")
                            nc.sync.dma_start(
                                out=RTt[:, 0:nl * 128],
                                in_=RT_in[:, (c * TPC) * 128:(c * TPC + nl) * 128])
                        ght = gp.tile([128, TPC * 128], F16, tag="ght")
                        ght3g = ght[:].rearrange("p (a k) -> p a k", k=128)
                        for (j0, ntl, base, hi_row) in gathers[c]:
                            hi = min(hi_row, TBL_ROWS)
                            cb = c * (CHUNK // 16)
                            nc.gpsimd.dma_gather(
                                ght3g[:, j0:j0 + ntl, :],
                                table[base:hi, :],
                                idx_s[:, cb + j0 * 8:cb + (j0 + ntl) * 8],
                                ntl * 128, ntl * 128, 128, elem_step=128,
                                queue_num=gq % 4)
                            gq += 1
                        if not live:
                            continue
                        ght3 = ght[:].rearrange("p (a k) -> p a k", k=128)
                        # per-edge a_dst via PE gather: psa = RT_j^T @ adw_w
                        psa = pap.tile([128, nl * H], F32, tag="a", name=f"psa{c}")
                        for j in live:
                            w = tile_w[c * TPC + j]
                            nc.tensor.matmul(
                                out=psa[:, j * H:(j + 1) * H],
                                lhsT=RTt[:, j * 128:(j + 1) * 128],
                                rhs=adwl[:, w * H:(w + 1) * H],
                                start=True, stop=True)
                        ew = mp.tile([128, nl * H], F32, tag="ew", name=f"ew{c}")
                        nc.vector.tensor_tensor(
                            out=ew[:].rearrange("p (a h) -> p a h", h=H),
                            in0=psa[:].rearrange("p (a h) -> p a h", h=H),
                            in1=ght3[:, 0:nl, D:D + H],
                            op=mybir.AluOpType.add)
                        lr = mp.tile([128, nl * H], F32, tag="lr", name=f"lr{c}")
                        nc.vector.tensor_tensor(
                            out=lr[:], in0=ew[:],
                            in1=cNEG[:].to_broadcast([128, nl * H]),
                            op=mybir.AluOpType.mult)
                        nc.vector.tensor_tensor(out=lr[:], in0=lr[:], in1=ew[:],
                                                op=mybir.AluOpType.max)
                        # exp, pre-expanded across the C dim (Act) -> DVE mult
                        we16 = mp.tile([128, nl * D], F16, tag="we", name=f"we{c}")
                        we4 = we16[:].rearrange("p (a h k) -> p a h k", h=H, k=CH)
                        nc.scalar.activation(
                            out=we4,
                            in_=lr[:].rearrange("p (a h) -> p a h", h=H)[:, :, :, None]
                                .to_broadcast([128, nl, H, CH]),
                            func=ACT.Exp)
                        msgt = mp.tile([128, nl * EC], F16, tag="msg", name=f"msg{c}")
                        msgt3 = msgt[:].rearrange("p (a k) -> p a k", k=EC)
                        # denominator cols written directly by a second exp (Act)
                        nc.scalar.activation(
                            out=msgt3[:, :, D:D + H],
                            in_=lr[:].rearrange("p (a h) -> p a h", h=H),
                            func=ACT.Exp)
                        nc.vector.tensor_tensor(
                            out=msgt3[:, :, 0:D].rearrange("p a (h k) -> p a h k", k=CH),
                            in0=ght3[:, 0:nl, 0:D].rearrange("p a (h k) -> p a h k", k=CH),
                            in1=we4,
                            op=mybir.AluOpType.mult)
                        for j in live:
                            t = c * TPC + j
                            w = tile_w[t]
                            if tile_first[t]:
                                group_ps[w] = pgp.tile([128, EC], F32, tag="g", name=f"grp{w}")
                            ps = group_ps[w]
                            nc.tensor.matmul(
                                out=ps[:], lhsT=Rt[:, j * 128:(j + 1) * 128],
                                rhs=msgt[:, j * EC:(j + 1) * EC],
                                start=tile_first[t], stop=tile_last[t])
                            if tile_last[t]:
                                nc.vector.tensor_tensor(
                                    out=acc[:, w * EC:(w + 1) * EC],
                                    in0=acc[:, w * EC:(w + 1) * EC],
                                    in1=ps[:], op=mybir.AluOpType.add)
                                del group_ps[w]
                                if tile_final[t]:
                                    if layer == 1:
                                        fin1(w, fwp, fpp)
                                    else:
                                        fin2(w, fwp)
                    assert not group_ps

            for rep in range(reps):
                nc.vector.memset(acc1[:], 0.0)
                nc.vector.memset(acc2[:], 0.0)
                # ================= layer 1 =================
                if rep == 0:
                    MARKS.append(("layer1", nc.next_id()))
                layer_run(1, rep)
                if rep == 0:
                    MARKS.append(("collective", nc.next_id()))
                if no_collective:
                    with tc.tile_pool(name=f"agcr{rep}", bufs=2) as acp:
                        for cc in range(NCORES):
                            for q in range(4):
                                t_ = acp.tile([D, 25 * 128], BF16, tag="agc")
                                w_ = (QB[q + 1] - QB[q]) * 128
                                nc.sync.dma_start(out=t_[:, 0:w_], in_=ag_in[q][:])
                                nc.sync.dma_start(
                                    out=ag_out[q][cc * D:(cc + 1) * D, :],
                                    in_=t_[:, 0:w_])
                else:
                    for q in range(4):
                        nc.gpsimd.collective_compute(
                            "AllGather", mybir.AluOpType.bypass,
                            ins=[ag_in[q][:]], outs=[ag_out[q][:]],
                            replica_groups=[list(range(NCORES))])

                # ================= layer 2 =================
                if rep == 0:
                    MARKS.append(("layer2", nc.next_id()))
                layer_run(2, rep)
    return nc


def make_inputs(edge_index, emb, W1, a_src1, a_dst1, b1, W2, a_src2, a_dst2, b2):
    NW, NPAD, NBUCK, TBL_ROWS = _derived()
    sched, idx_h, R_h, RT_h = prep(edge_index)

    W1 = np.asarray(W1, np.float32)
    a_s1 = np.asarray(a_src1, np.float32)
    a_d1 = np.asarray(a_dst1, np.float32)
    As = np.zeros((D, H1), np.float32)
    Ad = np.zeros((D, H1), np.float32)
    for h in range(H1):
        As[h * C1:(h + 1) * C1, h] = a_s1[h]
        Ad[h * C1:(h + 1) * C1, h] = a_d1[h]
    # [Waux | Wad] = [W | W@As | W@Ad]
    w1c = np.concatenate([W1, W1 @ As, W1 @ Ad], 1).astype(NPBF16)
    W2 = np.asarray(W2, np.float32)
    w2c = np.concatenate([W2, W2 @ np.asarray(a_src2, np.float32).T,
                          W2 @ np.asarray(a_dst2, np.float32).T], 1).astype(NPBF16)

    embT = np.zeros((D, NPAD), NPBF16)
    embT[:, :N] = np.asarray(emb, np.float32).T.astype(NPBF16)
    ident = np.eye(128, dtype=np.float32)
    b1t = np.broadcast_to(np.asarray(b1, np.float32)[None, :], (128, D)).copy()
    b2t = np.broadcast_to(np.asarray(b2, np.float32)[None, :], (128, D)).copy()

    in_maps = []
    for c in range(NCORES):
        in_maps.append({
            "embT": embT, "embTo": np.ascontiguousarray(embT[:, c * OWN:(c + 1) * OWN]),
            "w1c": w1c, "w2c": w2c,
            "b1t": b1t, "b2t": b2t, "ident": ident,
            "idx16": idx_h[c], "Rh": R_h[c], "RTh": RT_h[c],
        })
    return sched, in_maps


def kernel(edge_index, emb, W1, a_src1, a_dst1, b1, W2, a_src2, a_dst2, b2):
    sched, in_maps = make_inputs(edge_index, emb, W1, a_src1, a_dst1, b1,
                                 W2, a_src2, a_dst2, b2)
    nc = build(sched)
    nc.finalize()
    res = run_bass_kernel_spmd(nc, in_maps, core_ids=list(range(NCORES)))
    out = np.zeros((N, D), np.float32)
    for c in range(NCORES):
        lo, hi = c * OWN, min((c + 1) * OWN, N)
        if lo < N:
            out[lo:hi] = res.results[c]["out_own"][:hi - lo]
    return out


# revision 28
# speedup vs baseline: 1.2586x; 1.0859x over previous
"""Bass/Trainium2 kernel for the 2-layer GAT (nn_GAT_11106785427688).

Strategy (8 NeuronCores, SPMD single NEFF):
- dst-ownership sharding: core c owns nodes [c*OWN, (c+1)*OWN); it receives
  every edge whose dst it owns (~137K edges), so segment-softmax denominators
  and message sums complete locally -- no all-reduce. One AllGather of the
  layer-1 activations between layers; host assembles the final output from
  per-core slices.
- Per-edge gather of packed [h | a_src.h] rows (fp16, 256B) from an HBM table
  via the SWDGE dma_gather custom op (int16 indices -> src buckets of 32768
  rows; table rows permuted so the dense phase writes 2KB-contiguous runs).
- No indexed scatter (HW dma_scatter_add loses duplicate updates): edges are
  grouped by 128-node dst window; the one-hot R [edges x nodes] and its
  transpose RT [nodes x edges] are PRECOMPUTED ON HOST (pure edge-index
  preprocessing) and streamed from HBM, so the DVE never builds one-hots.
  R turns segment-sum into PE matmul accumulated in PSUM; RT gathers the
  per-window a_dst values to edges via PE. Softmax division is deferred:
  out = (sum_e w*h[src]) / (sum_e w).
- exp(leakyrelu(e)) computed without max-subtraction (shift-invariant).
- adw_fill (self-loops + per-window a_dst) is interleaved with the dense
  table build so PE/Act/DVE/DMA overlap instead of running serial phases.
"""
import numpy as np
import ml_dtypes

from concourse import bacc, mybir
import concourse.tile as tile
from concourse.bass_utils import run_bass_kernel_spmd

# ---- problem constants ----
N = 100000
D = 64
H1, C1 = 4, 16
NEG = 0.2
NCORES = 8
OWN = 12544                 # 98 windows * 128 per core
BUCK = 32768
CHUNK = 1024                # gather idxs per dma_gather call (ring limit)
TPC = CHUNK // 128          # tiles per chunk = 8

F16 = mybir.dt.float16
F32 = mybir.dt.float32
BF16 = mybir.dt.bfloat16
I16 = mybir.dt.int16
NPF16 = np.float16
NPBF16 = ml_dtypes.bfloat16

ACT = mybir.ActivationFunctionType


QBW = [0, 25, 50, 74, 98]       # window-quarter boundaries (AG split)


def _derived():
    NW = OWN // 128
    NPAD = NCORES * OWN
    NBUCK = (NPAD + BUCK - 1) // BUCK
    TBL_ROWS = NBUCK * BUCK
    return NW, NPAD, NBUCK, TBL_ROWS


def _nodeperm():
    """New table position of each padded-global node: quarter-major
    (q, core, window) so layer-2 table blocks depend on one AG quarter."""
    NW, NPAD, _, _ = _derived()
    pos = np.empty(NPAD, np.int64)
    p = 0
    for q in range(4):
        for co in range(NCORES):
            n0 = co * OWN + QBW[q] * 128
            cnt = (QBW[q + 1] - QBW[q]) * 128
            pos[n0:n0 + cnt] = np.arange(p, p + cnt)
            p += cnt
    return pos


def _perm_row(src):
    """Permuted table row for node src: tb*1024 + p*8 + j (write-friendly)."""
    tb, r = np.divmod(src, 1024)
    j, p = np.divmod(r, 128)
    return tb * 1024 + p * 8 + j


def prep(edge_index):
    """Vectorized host prep: quantile-banded schedule.

    Per-(core,window) edges sorted by src, quantile-spread into the padded
    window group (G_w = roundup128(max-over-cores)). Window tiles are split
    into bands of <=3 tiles; the schedule is band-major so consecutive tiles
    cover the same src-quantile region. Each 1024-slot chunk then spans <=~31
    perm-blocks and gets ONE dma_gather call with a dynamic host-computed
    base (int16 idx). Bands are chunk-aligned (pad tiles trail per band).

    Also builds, per core, the fp16 one-hot streams R (edge-major: used as
    matmul lhsT for the per-window segment sums) and RT (node-major: used as
    lhsT to gather per-window a_dst values to edge positions).
    """
    NW, NPAD, NBUCK, TBL_ROWS = _derived()
    pos = _nodeperm()
    # self-loops are handled densely in adw_fill, not in the gather sweep
    src = np.asarray(edge_index[0])
    dst = np.asarray(edge_index[1])
    owner = dst // OWN

    per_core = []
    counts = np.zeros((NCORES, NW), np.int64)
    for c in range(NCORES):
        m = owner == c
        s = pos[src[m]]               # table positions, quarter-major order
        d = dst[m] - c * OWN
        w = d >> 7
        order = np.lexsort((s, w))
        s, d, w = s[order], d[order], w[order]
        per_core.append((s, d, w))
        counts[c] = np.bincount(w, minlength=NW)

    gsize = ((counts.max(0) + 127) // 128 * 128).astype(np.int64)   # [NW]
    kw = gsize // 128                                               # tiles/window

    # band-major tile schedule: band b = quantile quarter [b/4,(b+1)/4) of
    # every window, so run centers align across windows regardless of K_w
    NBANDS = 4
    kb = [[int(round(b * int(kw[w]) / NBANDS)) for b in range(NBANDS + 1)]
          for w in range(NW)]
    tile_list = []          # (w, k) in schedule order
    for b in range(NBANDS):
        for w in range(NW):
            for k in range(kb[w][b], kb[w][b + 1]):
                tile_list.append((w, k))
        # chunk-align each band (pad tiles trail inside the band's last chunk)
        while len(tile_list) % TPC != 0:
            tile_list.append((-1, -1))

    n_tiles = len(tile_list)
    total_slots = n_tiles * 128
    n_chunks = total_slots // CHUNK
    tile_w = np.array([w for w, _ in tile_list], np.int64)
    # slot base of each (w,k) tile
    tile_base = {}
    for t, (w, k) in enumerate(tile_list):
        if w >= 0:
            tile_base[(w, k)] = t * 128
    # first/last per (window, band) run
    tile_first = np.zeros(n_tiles, bool)
    tile_last = np.zeros(n_tiles, bool)
    tile_final = np.zeros(n_tiles, bool)
    for t, (w, k) in enumerate(tile_list):
        if w < 0:
            continue
        tile_first[t] = k in [kb[w][b] for b in range(NBANDS)]
        tile_last[t] = (k + 1) in [kb[w][b + 1] for b in range(NBANDS)]
        tile_final[t] = k + 1 == int(kw[w])

    # per-core slot arrays + per-tile block ranges
    idx_h = np.zeros((NCORES, 128, n_chunks * (CHUNK // 16)), np.int16)
    R_h = np.zeros((NCORES, 128, n_tiles * 128), NPF16)
    RT_h = np.zeros((NCORES, 128, n_tiles * 128), NPF16)
    pr_all = np.zeros((NCORES, total_slots), np.int64)
    off_all = np.full((NCORES, total_slots), -1, np.int64)
    tb_arr = np.full(NW * 32, -1, np.int64)
    for (w, k), sb in tile_base.items():
        tb_arr[w * 32 + k] = sb
    kidx = np.arange(128)
    for c in range(NCORES):
        s, d, w = per_core[c]
        grp_first = np.searchsorted(w, np.arange(NW))
        rank = np.arange(len(s)) - grp_first[w]
        q = (rank * gsize[w]) // counts[c][w]      # quantile-spread in window
        slot = tb_arr[w * 32 + (q // 128)] + (q % 128)
        assert (slot >= 0).all()
        pr_all[c][slot] = _perm_row(s)
        off_all[c][slot] = d & 127
        offs = off_all[c].reshape(n_tiles, 128)
        # R[p, t*128+k] = (off(slot t*128+p) == k); pads (off=-1) -> zero col
        R_h[c] = (offs[:, :, None] == kidx[None, None, :]) \
            .transpose(1, 0, 2).reshape(128, -1).astype(NPF16)
        # RT[p, t*128+e] = (off(slot t*128+e) == p)
        RT_h[c] = (offs[None, :, :] == kidx[:, None, None]) \
            .reshape(128, -1).astype(NPF16)

    # per-chunk gather calls with dynamic base (split if span > 31 blocks)
    real = off_all >= 0
    blk = np.where(real, pr_all // 1024, 1 << 30)
    blk_hi = np.where(real, pr_all // 1024, -1)
    gathers = []
    slot_base = np.zeros(total_slots, np.int64)
    for cidx in range(n_chunks):
        calls = []
        j = 0
        nlive = sum(1 for jj in range(TPC) if tile_w[cidx * TPC + jj] >= 0)
        while j < nlive:
            j0 = j
            s0 = cidx * CHUNK + j0 * 128
            lo = int(blk[:, s0:s0 + 128].min())
            hi = int(blk_hi[:, s0:s0 + 128].max())
            j += 1
            while j < nlive:
                s1 = cidx * CHUNK + j * 128
                nlo = min(lo, int(blk[:, s1:s1 + 128].min()))
                nhi = max(hi, int(blk_hi[:, s1:s1 + 128].max()))
                if nhi - nlo > 31:
                    break
                lo, hi = nlo, nhi
                j += 1
            if lo >= (1 << 30):
                lo = 0
                hi = 0
            base = lo * 1024
            hi_row = min((hi + 1) * 1024, base + 32768)
            calls.append((j0, j - j0, int(base), int(hi_row)))
            slot_base[cidx * CHUNK + j0 * 128: cidx * CHUNK + j * 128] = base
        if not calls:
            calls.append((0, TPC, 0, 1024))
        gathers.append(calls)

    for c in range(NCORES):
        gi = pr_all[c] - slot_base
        gi[~real[c]] = 0
        assert (gi >= 0).all() and (gi < 32768).all()
        gia = gi.reshape(n_chunks, CHUNK // 16, 16).transpose(0, 2, 1)
        idx_h[c] = np.tile(gia, (1, 8, 1)).transpose(1, 0, 2).reshape(128, -1)

    sched = dict(n_chunks=n_chunks, tile_w=tile_w.tolist(),
                 tile_first=tile_first.tolist(), tile_last=tile_last.tolist(),
                 tile_final=tile_final.tolist(), gathers=gathers)
    return sched, idx_h, R_h, RT_h


MARKS = []


def build(sched, debug=False, no_collective=False, reps=1):
    MARKS.clear()
    NW, NPAD, NBUCK, TBL_ROWS = _derived()
    n_chunks = sched["n_chunks"]
    tile_w = sched["tile_w"]
    tile_first = sched["tile_first"]
    tile_last = sched["tile_last"]
    tile_final = sched["tile_final"]
    gathers = sched["gathers"]
    n_tiles = n_chunks * TPC
    NT_DENSE = NPAD // 128
    NB_DENSE = (NT_DENSE + 7) // 8

    nc = bacc.Bacc(None, target_bir_lowering=False, num_swdge_queues=4)

    embT = nc.dram_tensor("embT", [D, NPAD], BF16, kind="ExternalInput")
    embTo = nc.dram_tensor("embTo", [D, OWN], BF16, kind="ExternalInput")
    # wNc = [Waux | Wad] so adw needs a single matmul per window
    w1c_in = nc.dram_tensor("w1c", [D, D + 2 * H1], BF16, kind="ExternalInput")
    w2c_in = nc.dram_tensor("w2c", [D, D + 2], BF16, kind="ExternalInput")
    b1t_in = nc.dram_tensor("b1t", [128, D], F32, kind="ExternalInput")
    b2t_in = nc.dram_tensor("b2t", [128, D], F32, kind="ExternalInput")
    ident_in = nc.dram_tensor("ident", [128, 128], F32, kind="ExternalInput")
    idx_in = nc.dram_tensor("idx16", [128, n_chunks * (CHUNK // 16)], I16, kind="ExternalInput")
    R_in = nc.dram_tensor("Rh", [128, n_tiles * 128], F16, kind="ExternalInput")
    RT_in = nc.dram_tensor("RTh", [128, n_tiles * 128], F16, kind="ExternalInput")
    out_own = nc.dram_tensor("out_own", [OWN, D], F32, kind="ExternalOutput")

    table = nc.dram_tensor("table", [TBL_ROWS, 128], F16)
    # window-quarter split of the activation exchange so the AllGather
    # pipelines behind sweep1's tail and dense2 starts on quarter 0
    QB = [0, 25, 50, 74, NW]
    ag_in = [nc.dram_tensor(f"ag_in{q}", [D, (QB[q + 1] - QB[q]) * 128], BF16)
             for q in range(4)]
    ag_out = [nc.dram_tensor(f"ag_out{q}", [NCORES * D, (QB[q + 1] - QB[q]) * 128],
                             BF16, addr_space="Shared")
              for q in range(4)]

    # quarter-major run table: (start_pos, q, core, count)
    AG_RUNS = []
    _p = 0
    for _q in range(4):
        for _co in range(NCORES):
            _cnt = (QB[_q + 1] - QB[_q]) * 128
            AG_RUNS.append((_p, _q, _co, _cnt))
            _p += _cnt

    def q_of(w):
        for q in range(4):
            if w < QB[q + 1]:
                return q
        raise AssertionError

    def ag_in_slice(w0, w1):
        """Split window range [w0, w1) at quarter boundaries ->
        (tensor, col0, col1, out_off) pieces."""
        pieces = []
        w = w0
        while w < w1:
            q = q_of(w)
            we = min(w1, QB[q + 1])
            pieces.append((q, (w - QB[q]) * 128, (we - QB[q]) * 128,
                           (w - w0) * 128))
            w = we
        return pieces

    with tile.TileContext(nc) as tc:
        with tc.tile_pool(name="persist", bufs=1) as pp:
            b1t = pp.tile([128, D], F32)
            b2t = pp.tile([128, D], F32)
            ident = pp.tile([128, 128], F32)
            w1c = pp.tile([D, D + 2 * H1], BF16)
            w2c = pp.tile([D, D + 2], BF16)
            idx_s = pp.tile([128, n_chunks * (CHUNK // 16)], I16)
            adw = pp.tile([128, NW * H1], F16)
            adw2 = pp.tile([128, NW], F16)
            acc1 = pp.tile([128, NW * (D + H1)], F32)
            acc2 = pp.tile([128, NW * (D + 1)], F32)
            # broadcast constants: tensor_tensor + [128,1] const is far
            # cheaper than TENSOR_SCALAR with an immediate (~1.6us fixed)
            cNEG = pp.tile([128, 1], F32)
            cEPS = pp.tile([128, 1], F32)
            cNRM = pp.tile([128, 1], F32)
            nc.vector.memset(cNEG[:], NEG)
            nc.vector.memset(cEPS[:], 1e-16)
            nc.vector.memset(cNRM[:], 1e-12)
            for t_, s_ in [(b1t, b1t_in), (b2t, b2t_in), (ident, ident_in),
                           (w1c, w1c_in), (w2c, w2c_in),
                           (idx_s, idx_in)]:
                nc.sync.dma_start(out=t_[:], in_=s_[:])

            def fin1(w, fp, fpp):
                """Finalize window w of layer 1: softmax div, bias, ELU,
                transpose, write ag_in column block."""
                EC = D + H1
                den = fp.tile([128, H1], F32, tag="den")
                nc.vector.tensor_tensor(
                    out=den[:], in0=acc1[:, w * EC + D:(w + 1) * EC],
                    in1=cEPS[:].to_broadcast([128, H1]), op=mybir.AluOpType.add)
                rec = fp.tile([128, H1], F32, tag="rec")
                nc.vector.reciprocal(out=rec[:], in_=den[:])
                x2 = fp.tile([128, D], F32, tag="x2")
                nc.vector.tensor_tensor(
                    out=x2[:].rearrange("p (h k) -> p h k", k=C1),
                    in0=acc1[:, w * EC:w * EC + D].rearrange("p (h k) -> p h k", k=C1),
                    in1=rec[:, :, None].to_broadcast([128, H1, C1]),
                    op=mybir.AluOpType.mult)
                nc.vector.tensor_tensor(out=x2[:], in0=x2[:], in1=b1t[:],
                                        op=mybir.AluOpType.add)
                # elu(x) = relu(x) - relu(1 - exp(x))
                ex = fp.tile([128, D], F32, tag="ex")
                nc.scalar.activation(out=ex[:], in_=x2[:], func=ACT.Exp)
                u = fp.tile([128, D], F32, tag="u")
                nc.scalar.activation(out=u[:], in_=ex[:], func=ACT.Relu,
                                     scale=-1.0, bias=1.0)
                r = fp.tile([128, D], F32, tag="r")
                nc.scalar.activation(out=r[:], in_=x2[:], func=ACT.Relu)
                xe = fp.tile([128, D], F32, tag="xe")
                nc.vector.tensor_tensor(out=xe[:], in0=r[:], in1=u[:],
                                        op=mybir.AluOpType.subtract)
                pst = fpp.tile([D, 128], F32, tag="t")
                nc.tensor.transpose(out=pst[:], in_=xe[:], identity=ident[:])
                xt = fp.tile([D, 128], BF16, tag="xt")
                nc.scalar.activation(out=xt[:], in_=pst[:], func=ACT.Copy)
                q = q_of(w)
                c0 = (w - QB[q]) * 128
                nc.sync.dma_start(out=ag_in[q][:, c0:c0 + 128], in_=xt[:])

            def fin2(w, fp):
                """Finalize window w of layer 2: softmax div, bias, l2-norm,
                write out_own rows."""
                EC = D + 1
                den = fp.tile([128, 1], F32, tag="den")
                nc.vector.tensor_tensor(
                    out=den[:], in0=acc2[:, w * EC + D:(w + 1) * EC],
                    in1=cEPS[:], op=mybir.AluOpType.add)
                rec = fp.tile([128, 1], F32, tag="rec")
                nc.vector.reciprocal(out=rec[:], in_=den[:])
                o = fp.tile([128, D], F32, tag="o")
                nc.vector.tensor_tensor(
                    out=o[:], in0=acc2[:, w * EC:w * EC + D],
                    in1=rec[:].to_broadcast([128, D]), op=mybir.AluOpType.mult)
                nc.vector.tensor_tensor(out=o[:], in0=o[:], in1=b2t[:],
                                        op=mybir.AluOpType.add)
                sq = fp.tile([128, D], F32, tag="sq")
                ss = fp.tile([128, 1], F32, tag="ss")
                nc.vector.scalar_tensor_tensor(
                    out=sq[:], in0=o[:], scalar=1.0, in1=o[:],
                    op0=mybir.AluOpType.mult, op1=mybir.AluOpType.mult,
                    accum_out=ss[:])
                nrm = fp.tile([128, 1], F32, tag="nr")
                nc.scalar.activation(out=nrm[:], in_=ss[:], func=ACT.Sqrt)
                nc.vector.tensor_tensor(out=nrm[:], in0=nrm[:], in1=cNRM[:],
                                        op=mybir.AluOpType.max)
                rn = fp.tile([128, 1], F32, tag="rn")
                nc.vector.reciprocal(out=rn[:], in_=nrm[:])
                of = fp.tile([128, D], F32, tag="of")
                nc.vector.tensor_tensor(out=of[:], in0=o[:],
                                        in1=rn[:].to_broadcast([128, D]),
                                        op=mybir.AluOpType.mult)
                nc.sync.dma_start(out=out_own[w * 128:(w + 1) * 128, :], in_=of[:])

            def layer_run(layer, rep):
                """One layer: adw (self-loops + per-window a_dst), dense table
                build, and the edge sweep — all under coexisting pools so the
                sweep's early bands overlap the tail of the dense build."""
                H = H1 if layer == 1 else 1
                CH = C1 if layer == 1 else D
                EC = D + H
                acc = acc1 if layer == 1 else acc2
                adwl = adw if layer == 1 else adw2
                dst_t = adwl
                wcomb = w1c if layer == 1 else w2c
                waux = wcomb[:, 0:EC]
                with tc.tile_pool(name=f"aw{layer}r{rep}", bufs=3) as ap, \
                     tc.tile_pool(name=f"dns{layer}r{rep}", bufs=3) as dp, \
                     tc.tile_pool(name=f"eg{layer}r{rep}", bufs=10) as gp, \
                     tc.tile_pool(name=f"er{layer}r{rep}", bufs=8) as rp, \
                     tc.tile_pool(name=f"em{layer}r{rep}", bufs=6) as mp, \
                     tc.tile_pool(name=f"fw{layer}r{rep}", bufs=3) as fwp, \
                     tc.tile_pool(name=f"epa{layer}r{rep}", bufs=2, space="PSUM") as pap, \
                     tc.tile_pool(name=f"fwp{layer}r{rep}", bufs=2, space="PSUM") as fpp, \
                     tc.tile_pool(name=f"epg{layer}r{rep}", bufs=2, space="PSUM") as pgp:

                    # ---- adw: per-window a_dst + self-loop contribution ----
                    app = [None]

                    def adw_step(w, ltb):
                        lt = ltb[:, (w % 8) * 128:(w % 8 + 1) * 128]
                        psh = app[0].tile([128, EC + H], F32, tag="h")
                        nc.tensor.matmul(out=psh[:], lhsT=lt, rhs=wcomb[:],
                                         start=True, stop=True)
                        nc.scalar.activation(out=dst_t[:, w * H:(w + 1) * H],
                                             in_=psh[:, EC:EC + H], func=ACT.Copy)
                        ho = ap.tile([128, EC], F16, tag="h16")
                        nc.scalar.activation(out=ho[:], in_=psh[:, 0:EC],
                                             func=ACT.Copy)
                        es = ap.tile([128, H], F32, tag="es")
                        nc.vector.tensor_tensor(out=es[:], in0=ho[:, D:D + H],
                                                in1=dst_t[:, w * H:(w + 1) * H],
                                                op=mybir.AluOpType.add)
                        lrs = ap.tile([128, H], F32, tag="lrs")
                        nc.vector.tensor_tensor(
                            out=lrs[:], in0=es[:],
                            in1=cNEG[:].to_broadcast([128, H]),
                            op=mybir.AluOpType.mult)
                        nc.vector.tensor_tensor(out=lrs[:], in0=lrs[:], in1=es[:],
                                                op=mybir.AluOpType.max)
                        wx = ap.tile([128, D], F16, tag="wx")
                        wx3 = wx[:].rearrange("p (h k) -> p h k", k=CH)
                        nc.scalar.activation(
                            out=wx3,
                            in_=lrs[:, :, None].to_broadcast([128, H, CH]),
                            func=ACT.Exp)
                        ms = ap.tile([128, D], F32, tag="ms")
                        nc.vector.tensor_tensor(out=ms[:], in0=ho[:, 0:D], in1=wx[:],
                                                op=mybir.AluOpType.mult)
                        nc.vector.tensor_tensor(
                            out=acc[:, w * EC:w * EC + D],
                            in0=acc[:, w * EC:w * EC + D], in1=ms[:],
                            op=mybir.AluOpType.add)
                        nc.vector.tensor_tensor(
                            out=acc[:, w * EC + D:(w + 1) * EC],
                            in0=acc[:, w * EC + D:(w + 1) * EC],
                            in1=wx3[:, :, 0],
                            op=mybir.AluOpType.add)

                    # ---- dense: x @ Waux -> fp16 table rows ----
                    dpp = [None]

                    def dense_step(tb0):
                        nb = min(2, NB_DENSE - tb0)
                        lt = dp.tile([D, 2 * 1024], BF16, tag="lhs")
                        if layer == 1:
                            nc.sync.dma_start(
                                out=lt[:, 0:nb * 1024],
                                in_=embT[:, tb0 * 1024:(tb0 + nb) * 1024])
                        else:
                            # quarter-major position range -> (q, core) runs
                            p0, p1 = tb0 * 1024, (tb0 + nb) * 1024
                            for (rs, rq, rc, rcnt) in AG_RUNS:
                                lo = max(p0, rs)
                                hi = min(p1, rs + rcnt)
                                if lo >= hi:
                                    continue
                                nc.sync.dma_start(
                                    out=lt[:, lo - p0:hi - p0],
                                    in_=ag_out[rq][rc * D:(rc + 1) * D,
                                                   lo - rs:hi - rs])
                        stg = dp.tile([128, 2 * 1024], F16, tag="stg")
                        for j in range(8 * nb):
                            ps = dpp[0].tile([128, EC], F32, tag="d")
                            nc.tensor.matmul(out=ps[:], lhsT=lt[:, j * 128:(j + 1) * 128],
                                             rhs=waux, start=True, stop=True)
                            if j % 2 == 0:
                                nc.scalar.activation(
                                    out=stg[:, j * 128:j * 128 + EC], in_=ps[:],
                                    func=ACT.Copy)
                            else:
                                nc.vector.tensor_copy(
                                    out=stg[:, j * 128:j * 128 + EC], in_=ps[:])
                        for b in range(nb):
                            nc.sync.dma_start(
                                out=table[(tb0 + b) * 1024:(tb0 + b + 1) * 1024]
                                .rearrange("(p j) k -> p (j k)", j=8),
                                in_=stg[:, b * 1024:(b + 1) * 1024])

                    # dense first: the sweep's early bands wait on the table,
                    # so table blocks get scheduling priority over adw
                    with tc.tile_pool(name=f"dnp{layer}r{rep}", bufs=2,
                                      space="PSUM") as dpp_:
                        dpp[0] = dpp_
                        for tb0 in range(0, NB_DENSE, 2):
                            dense_step(tb0)

                    with tc.tile_pool(name=f"awp{layer}r{rep}", bufs=1,
                                      space="PSUM") as app_:
                        app[0] = app_
                        ltb = None
                        for w in range(NW):
                            if w % 8 == 0:
                                nwb = min(8, NW - w)
                                ltb = ap.tile([D, 8 * 128], BF16, tag="lb")
                                if layer == 1:
                                    nc.sync.dma_start(
                                        out=ltb[:, 0:nwb * 128],
                                        in_=embTo[:, w * 128:(w + nwb) * 128])
                                else:
                                    for (q, a0, a1, oo) in ag_in_slice(w, w + nwb):
                                        nc.sync.dma_start(
                                            out=ltb[:, oo:oo + (a1 - a0)],
                                            in_=ag_in[q][:, a0:a1])
                            adw_step(w, ltb)

                    # ---- edge sweep ----
                    group_ps = {}
                    gq = 0
                    for c in range(n_chunks):
                        live = [j for j in range(TPC) if tile_w[c * TPC + j] >= 0]
                        assert live == list(range(len(live))), "pads must trail"
                        nl = len(live)
                        # one-hot streams from host (R: edge-major, RT: node-major);
                        # emitted before the gather so they prefetch ahead
                        if live:
                            Rt = rp.tile([128, TPC * 128], F16, tag="R")
                            nc.sync.dma_start(
                                out=Rt[:, 0:nl * 128],
                                in_=R_in[:, (c * TPC) * 128:(c * TPC + nl) * 128])
                            RTt = rp.tile([128, TPC * 128], F16, tag="RT")
                            nc.sync.dma_start(
                                out=RTt[:, 0:nl * 128],
                                in_=RT_in[:, (c * TPC) * 128:(c * TPC + nl) * 128])
                        ght = gp.tile([128, TPC * 128], F16, tag="ght")
                        ght3g = ght[:].rearrange("p (a k) -> p a k", k=128)
                        for (j0, ntl, base, hi_row) in gathers[c]:
                            hi = min(hi_row, TBL_ROWS)
                            cb = c * (CHUNK // 16)
                            nc.gpsimd.dma_gather(
                                ght3g[:, j0:j0 + ntl, :],
                                table[base:hi, :],
                                idx_s[:, cb + j0 * 8:cb + (j0 + ntl) * 8],
                                ntl * 128, ntl * 128, 128, elem_step=128,
                                queue_num=gq % 4)
                            gq += 1
                        if not live:
                            continue
                        ght3 = ght[:].rearrange("p (a k) -> p a k", k=128)
                        # per-edge a_dst via PE gather: psa = RT_j^T @ adw_w
                        psa = pap.tile([128, nl * H], F32, tag="a", name=f"psa{c}")
                        for j in live:
                            w = tile_w[c * TPC + j]
                            nc.tensor.matmul(
                                out=psa[:, j * H:(j + 1) * H],
                                lhsT=RTt[:, j * 128:(j + 1) * 128],
                                rhs=adwl[:, w * H:(w + 1) * H],
                                start=True, stop=True)
                        ew = mp.tile([128, nl * H], F32, tag="ew", name=f"ew{c}")
                        nc.vector.tensor_tensor(
                            out=ew[:].rearrange("p (a h) -> p a h", h=H),
                            in0=psa[:].rearrange("p (a h) -> p a h", h=H),
                            in1=ght3[:, 0:nl, D:D + H],
                            op=mybir.AluOpType.add)
                        lr = mp.tile([128, nl * H], F32, tag="lr", name=f"lr{c}")
                        nc.vector.tensor_tensor(
                            out=lr[:], in0=ew[:],
                            in1=cNEG[:].to_broadcast([128, nl * H]),
                            op=mybir.AluOpType.mult)
                        nc.vector.tensor_tensor(out=lr[:], in0=lr[:], in1=ew[:],
                                                op=mybir.AluOpType.max)
                        # exp, pre-expanded across the C dim (Act) -> DVE mult
                        we16 = mp.tile([128, nl * D], F16, tag="we", name=f"we{c}")
                        we4 = we16[:].rearrange("p (a h k) -> p a h k", h=H, k=CH)
                        nc.scalar.activation(
                            out=we4,
                            in_=lr[:].rearrange("p (a h) -> p a h", h=H)[:, :, :, None]
                                .to_broadcast([128, nl, H, CH]),
                            func=ACT.Exp)
                        msgt = mp.tile([128, nl * EC], F16, tag="msg", name=f"msg{c}")
                        msgt3 = msgt[:].rearrange("p (a k) -> p a k", k=EC)
                        # denominator cols written directly by a second exp (Act)
                        nc.scalar.activation(
                            out=msgt3[:, :, D:D + H],
                            in_=lr[:].rearrange("p (a h) -> p a h", h=H),
                            func=ACT.Exp)
                        nc.vector.tensor_tensor(
                            out=msgt3[:, :, 0:D].rearrange("p a (h k) -> p a h k", k=CH),
                            in0=ght3[:, 0:nl, 0:D].rearrange("p a (h k) -> p a h k", k=CH),
                            in1=we4,
                            op=mybir.AluOpType.mult)
                        for j in live:
                            t = c * TPC + j
                            w = tile_w[t]
                            if tile_first[t]:
                                group_ps[w] = pgp.tile([128, EC], F32, tag="g", name=f"grp{w}")
                            ps = group_ps[w]
                            nc.tensor.matmul(
                                out=ps[:], lhsT=Rt[:, j * 128:(j + 1) * 128],
                                rhs=msgt[:, j * EC:(j + 1) * EC],
                                start=tile_first[t], stop=tile_last[t])
                            if tile_last[t]:
                                nc.vector.tensor_tensor(
                                    out=acc[:, w * EC:(w + 1) * EC],
                                    in0=acc[:, w * EC:(w + 1) * EC],
                                    in1=ps[:], op=mybir.AluOpType.add)
                                del group_ps[w]
                                if tile_final[t]:
                                    if layer == 1:
                                        fin1(w, fwp, fpp)
                                    else:
                                        fin2(w, fwp)
                    assert not group_ps

            for rep in range(reps):
                nc.vector.memset(acc1[:], 0.0)
                nc.vector.memset(acc2[:], 0.0)
                # ================= layer 1 =================
                if rep == 0:
                    MARKS.append(("layer1", nc.next_id()))
                layer_run(1, rep)
                if rep == 0:
                    MARKS.append(("collective", nc.next_id()))
                if no_collective:
                    with tc.tile_pool(name=f"agcr{rep}", bufs=2) as acp:
                        for cc in range(NCORES):
                            for q in range(4):
                                t_ = acp.tile([D, 25 * 128], BF16, tag="agc")
                                w_ = (QB[q + 1] - QB[q]) * 128
                                nc.sync.dma_start(out=t_[:, 0:w_], in_=ag_in[q][:])
                                nc.sync.dma_start(
                                    out=ag_out[q][cc * D:(cc + 1) * D, :],
                                    in_=t_[:, 0:w_])
                else:
                    for q in range(4):
                        nc.gpsimd.collective_compute(
                            "AllGather", mybir.AluOpType.bypass,
                            ins=[ag_in[q][:]], outs=[ag_out[q][:]],
                            replica_groups=[list(range(NCORES))])

                # ================= layer 2 =================
                if rep == 0:
                    MARKS.append(("layer2", nc.next_id()))
                layer_run(2, rep)
    return nc


def make_inputs(edge_index, emb, W1, a_src1, a_dst1, b1, W2, a_src2, a_dst2, b2):
    NW, NPAD, NBUCK, TBL_ROWS = _derived()
    sched, idx_h, R_h, RT_h = prep(edge_index)

    W1 = np.asarray(W1, np.float32)
    a_s1 = np.asarray(a_src1, np.float32)
    a_d1 = np.asarray(a_dst1, np.float32)
    As = np.zeros((D, H1), np.float32)
    Ad = np.zeros((D, H1), np.float32)
    for h in range(H1):
        As[h * C1:(h + 1) * C1, h] = a_s1[h]
        Ad[h * C1:(h + 1) * C1, h] = a_d1[h]
    # [Waux | Wad] = [W | W@As | W@Ad]
    w1c = np.concatenate([W1, W1 @ As, W1 @ Ad], 1).astype(NPBF16)
    W2 = np.asarray(W2, np.float32)
    w2c = np.concatenate([W2, W2 @ np.asarray(a_src2, np.float32).T,
                          W2 @ np.asarray(a_dst2, np.float32).T], 1).astype(NPBF16)

    embT0 = np.zeros((D, NPAD), NPBF16)
    embT0[:, :N] = np.asarray(emb, np.float32).T.astype(NPBF16)
    pos = _nodeperm()
    embT = np.empty_like(embT0)
    embT[:, pos] = embT0
    ident = np.eye(128, dtype=np.float32)
    b1t = np.broadcast_to(np.asarray(b1, np.float32)[None, :], (128, D)).copy()
    b2t = np.broadcast_to(np.asarray(b2, np.float32)[None, :], (128, D)).copy()

    in_maps = []
    for c in range(NCORES):
        in_maps.append({
            "embT": embT, "embTo": np.ascontiguousarray(embT0[:, c * OWN:(c + 1) * OWN]),
            "w1c": w1c, "w2c": w2c,
            "b1t": b1t, "b2t": b2t, "ident": ident,
            "idx16": idx_h[c], "Rh": R_h[c], "RTh": RT_h[c],
        })
    return sched, in_maps


def kernel(edge_index, emb, W1, a_src1, a_dst1, b1, W2, a_src2, a_dst2, b2):
    sched, in_maps = make_inputs(edge_index, emb, W1, a_src1, a_dst1, b1,
                                 W2, a_src2, a_dst2, b2)
    nc = build(sched)
    nc.finalize()
    res = run_bass_kernel_spmd(nc, in_maps, core_ids=list(range(NCORES)))
    out = np.zeros((N, D), np.float32)
    for c in range(NCORES):
        lo, hi = c * OWN, min((c + 1) * OWN, N)
        if lo < N:
            out[lo:hi] = res.results[c]["out_own"][:hi - lo]
    return out
